# revision 13
# baseline (speedup 1.0000x reference)
"""Trainium2 Bass kernel for nn_BasicBlock (ReActNet-style binary basic block).

Strategy: data-parallel over batch (8 images -> 8 NeuronCores). All compute for
one image is local except the BatchNorm batch statistics, which are reduced
across cores with two tiny (128x2 fp32) AllReduces inside a single kernel launch.

Per-core layout ("folded"): the 64-channel image is split into two height-halves
stacked on the 128 SBUF partitions (partition c = channel c rows 0..63,
partition 64+c = channel c rows 64..127). The 3x3 conv input is stored padded as
66 rows x 130 cols per partition (1-pixel halo); both halves share identical
access-pattern offsets, so one matmul stream computes both halves using
block-diagonal (duplicated) weights.

Binary ops are exact in bf16: activations are sign() in {-1,0,+1}; weights are
alpha*sign(w) with per-output-channel alpha. bf16 rounding of alpha is a pure
per-channel scale, which BatchNorm divides right back out, so conv in bf16 with
fp32 PSUM accumulation reproduces the fp32 reference exactly.

Bias/PReLU algebra is folded into per-channel scalars so the whole post-conv
chain is 4 fused passes:
  u = (c * g') + residual          [DVE scalar_tensor_tensor]
  r = Relu(u + Bp)                 [ACT, per-partition bias]
  q = (u * p) + (p*Bp [+ b23])     [DVE tensor_scalar dual-op]
  M = (r * (1-p)) + q              [GPSIMD scalar_tensor_tensor]
which equals prelu(u + Bp) [+ b23] since prelu(z) = (1-p)*relu(z) + p*z.
"""

import functools
import numpy as np

import concourse.bass as bass
import concourse.bacc as bacc
import concourse.tile as tile
from concourse import mybir

EPS = 1e-5
B, C, H = 8, 64, 128
NCORES = 8
PAD = 130          # padded row length (128 + 2)
ROWS = 66          # padded rows per half (64 + 2 halo)
XF = ROWS * PAD    # 8580 padded elements per partition
NF = 64 * 128      # 8192 dense elements per partition (half image)
NTOT = B * H * H   # 131072: count per channel for batch stats
TAPS = [(dy, dx) for dy in range(3) for dx in range(3)]

f32 = mybir.dt.float32
bf16 = mybir.dt.bfloat16
Alu = mybir.AluOpType
Act = mybir.ActivationFunctionType


def _post_bn_math(nc, sm, ar, gamma_col, beta_col, alpha_col, alphasq_col):
    """Tiny per-partition ([128,1]) math turning AllReduced integer count
    sums into per-channel scalars. The conv ran with pure sign(+-1) weights,
    so sums are exact integer counts; alpha is applied here in fp32.

    Writes sm cols: 2 = muk (mean in count units), 19 = G (gamma*rsqrt*alpha).
    The apply pass computes (k - muk)*G == gamma*(c - mu)*rsqrt(var+eps),
    matching the reference's operation order.
    """
    TT = nc.vector.tensor_tensor
    TS = nc.vector.tensor_scalar

    def c(i):
        return sm[:, i:i + 1]

    TT(out=c(0), in0=ar[:, 0:1], in1=ar[:, 2:3], op=Alu.add)      # S1 counts
    TT(out=c(1), in0=ar[:, 1:2], in1=ar[:, 3:4], op=Alu.add)      # S2 counts
    TS(c(2), c(0), 1.0 / NTOT, None, Alu.mult)                    # muk
    TS(c(3), c(1), 1.0 / NTOT, None, Alu.mult)                    # E[k^2]
    TS(c(4), c(2), c(2), None, Alu.mult)                          # muk^2
    TT(out=c(5), in0=c(3), in1=c(4), op=Alu.subtract)             # vark
    TT(out=c(6), in0=c(5), in1=alphasq_col, op=Alu.mult)          # var
    TS(c(6), c(6), EPS, None, Alu.add)                            # var+eps
    nc.scalar.activation(out=c(7), in_=c(6), func=Act.Sqrt)       # ~sqrt
    nc.vector.reciprocal(out=c(8), in_=c(7))                      # y0 ~ rsqrt
    # one Newton step: y = y0*(1.5 - 0.5*(var+eps)*y0^2)
    TS(c(9), c(8), c(8), None, Alu.mult)                          # y0^2
    TT(out=c(10), in0=c(9), in1=c(6), op=Alu.mult)                # v*y0^2
    TS(c(11), c(10), -0.5, 1.5, Alu.mult, Alu.add)                # 1.5-0.5*
    TT(out=c(12), in0=c(8), in1=c(11), op=Alu.mult)               # y (rsqrt)
    TT(out=c(13), in0=c(12), in1=gamma_col, op=Alu.mult)          # g'
    TT(out=c(19), in0=c(13), in1=alpha_col, op=Alu.mult)          # G = g'*alpha
    # fold the additive bias beta into the subtracted mean so the residual
    # add needs no per-channel scalar: (k - (muk - beta/G))*G = (k-muk)*G+beta
    nc.vector.reciprocal(out=c(14), in_=c(19))                    # 1/G
    TT(out=c(15), in0=beta_col, in1=c(14), op=Alu.mult)           # beta/G
    TT(out=c(20), in0=c(2), in1=c(15), op=Alu.subtract)           # muk'


def build_nc():
    nc = bacc.Bacc(None, num_devices=NCORES)

    x_t = nc.declare_dram_parameter("x", [C, H, H], f32, isOutput=False)
    w1_t = nc.declare_dram_parameter("w1t", [128, 9 * 128], bf16, isOutput=False)
    w2_t = nc.declare_dram_parameter("w2t", [128, 256], bf16, isOutput=False)
    pv_t = nc.declare_dram_parameter("pvec", [128, 20], f32, isOutput=False)
    out_t = nc.declare_dram_parameter("out", [C, H, H], f32, isOutput=True)

    RG = [list(range(NCORES))]
    # row-chunk boundaries for chunked full-image passes
    RCH = [(0, 16), (16, 32), (32, 48), (48, 64)]
    HCH = [(0, 32), (32, 64)]

    with tile.TileContext(nc, num_cores=NCORES) as tc:
        with tc.tile_pool(name="main", bufs=1) as pool, \
             tc.tile_pool(name="pp", bufs=2, space="PSUM") as pp, \
             tc.tile_pool(name="dp", bufs=1, space="DRAM") as dp:

            w1s = pool.tile([128, 9 * 128], bf16, name="w1s")
            w2s = pool.tile([128, 256], bf16, name="w2s")
            pv = pool.tile([128, 20], f32, name="pv")
            nc.sync.dma_start(out=w1s[:, :], in_=w1_t[:, :])
            nc.sync.dma_start(out=w2s[:, :], in_=w2_t[:, :])
            nc.sync.dma_start(out=pv[:, :], in_=pv_t[:, :])

            # ---- load x into padded/folded layout -------------------------
            xpad = pool.tile([128, XF], f32, name="xpad", tag="A")
            xp3 = xpad.rearrange("p (r c) -> p r c", c=PAD)
            # zero borders: top halo row of half0, bottom halo row of half1,
            # left and right halo columns
            nc.vector.memset(xp3[0:64, 0:1, :], 0.0)
            nc.vector.memset(xp3[64:128, 65:66, :], 0.0)
            nc.vector.memset(xp3[:, :, 0:1], 0.0)
            nc.vector.memset(xp3[:, :, 129:130], 0.0)
            for (r0, r1) in [(0, 17), (17, 33), (33, 49), (49, 65)]:
                # half0 partitions: padded row r holds image row r-1 -> rows
                # 1..65 hold image rows 0..64 (incl. 1-row halo below)
                nc.sync.dma_start(out=xp3[0:64, 1 + r0:1 + r1, 1:129],
                                  in_=x_t[:, r0:r1, :])
                # half1 partitions: padded row r holds image row 63+r
                nc.sync.dma_start(out=xp3[64:128, r0:r1, 1:129],
                                  in_=x_t[:, 63 + r0:63 + r1, :])

            # ---- T1 = sign(x + b11), padded, bf16 -------------------------
            T1 = pool.tile([128, XF], bf16, name="T1", tag="B")
            T13 = T1.rearrange("p (r c) -> p r c", c=PAD)
            for (r0, r1) in [(0, 17), (17, 33), (33, 50), (50, 66)]:
                nc.scalar.activation(out=T13[:, r0:r1, :], in_=xp3[:, r0:r1, :],
                                     func=Act.Sign, bias=pv[:, 0:1])
            # restore zero borders (sign(0+b11) may be nonzero there)
            nc.vector.memset(T13[0:64, 0:1, :], 0.0)
            nc.vector.memset(T13[64:128, 65:66, :], 0.0)
            nc.vector.memset(T13[:, :, 0:1], 0.0)
            nc.vector.memset(T13[:, :, 129:130], 0.0)

            # ---- conv1: 9 accumulating matmuls per 512-col chunk ----------
            c1p = pool.tile([128, NF], f32, name="c1p", tag="C")
            s1cols = pool.tile([128, 16], f32, name="s1cols")
            pk1 = pool.tile([128, 2], f32, name="pk1")
            s1sq = pool.tile([128, 16], f32, name="s1sq")
            sqs = pool.tile([128, 2 * 512], bf16, name="sqs")
            for bg in range(4):
                pss = [pp.tile([128, 512], f32, name=f"ps{bg}_{j}", tag=f"ps{j}")
                       for j in range(4)]
                # taps outer: the stationary operand is reused across the 4
                # chunks of the group, so LDWEIGHTS amortizes and the PE gets
                # dense back-to-back matmul bursts (keeps HAM un-throttled)
                for t, (dy, dx) in enumerate(TAPS):
                    for j in range(4):
                        q = 4 * bg + j
                        nc.tensor.matmul(pss[j][:, :],
                                         lhsT=w1s[:, t * 128:(t + 1) * 128],
                                         rhs=T13[:, 4 * q + dy:4 * q + dy + 4, dx:dx + 128],
                                         start=(t == 0), stop=(t == 8),
                                         skip_group_check=True)
                for j in range(4):
                    q = 4 * bg + j
                    nc.scalar.activation(out=c1p[:, 512 * q:512 * (q + 1)],
                                         in_=pss[j][:, :], func=Act.Copy,
                                         bias=0.0, scale=1.0,
                                         accum_out=s1cols[:, q:q + 1])
                    nc.scalar.activation(out=sqs[:, 512 * (q % 2):512 * (q % 2 + 1)],
                                         in_=pss[j][:, :], func=Act.Square,
                                         accum_out=s1sq[:, q:q + 1])

            nc.vector.tensor_reduce(out=pk1[:, 1:2], in_=s1sq[:, :],
                                    axis=mybir.AxisListType.X, op=Alu.add)
            nc.vector.tensor_reduce(out=pk1[:, 0:1], in_=s1cols[:, :],
                                    axis=mybir.AxisListType.X, op=Alu.add)

            # ---- AllReduce branch-1 stats ---------------------------------
            cc1i = dp.tile([128, 2], f32, name="cc1i")
            cc1o = dp.tile([128, 2], f32, name="cc1o", addr_space="Shared")
            nc.gpsimd.dma_start(out=cc1i[:, :], in_=pk1[:, :])
            nc.gpsimd.collective_compute("AllReduce", Alu.add, replica_groups=RG,
                                         ins=[cc1i[:, :].opt()],
                                         outs=[cc1o[:, :].opt()])
            ar1 = pool.tile([128, 4], f32, name="ar1")
            cc1r = cc1o[:, :].rearrange("(h c) s -> c h s", h=2)
            nc.sync.dma_start(out=ar1[0:64, :].rearrange("c (h s) -> c h s", h=2),
                              in_=cc1r)
            nc.sync.dma_start(out=ar1[64:128, :].rearrange("c (h s) -> c h s", h=2),
                              in_=cc1r)

            sm1 = pool.tile([128, 24], f32, name="sm1")
            # pv col2 = gamma1, col3 = beta1+b12, col4 = p1
            _post_bn_math(nc, sm1, ar1, pv[:, 2:3], pv[:, 3:4], pv[:, 11:12], pv[:, 12:13])

            # ---- branch-1 apply: bn = (k-muk)*G; u = bn+beta12+x;
            #      M1 = prelu(u) = max(u, p1*u) ----------------------------
            w1b = pool.tile([128, NF], f32, name="w1b", tag="D")
            w13 = w1b.rearrange("p (r c) -> p r c", c=128)
            u = pool.tile([128, NF], f32, name="u", tag="E")
            u3 = u.rearrange("p (r c) -> p r c", c=128)
            q1 = pool.tile([128, NF], f32, name="q1", tag="C")
            M1 = pool.tile([128, NF], f32, name="M1", tag="A")
            M13 = M1.rearrange("p (r c) -> p r c", c=128)
            for (r0, r1) in HCH:
                sl = slice(128 * r0, 128 * r1)
                nc.vector.tensor_scalar(w1b[:, sl], c1p[:, sl],
                                        sm1[:, 20:21], sm1[:, 19:20],
                                        Alu.subtract, Alu.mult)
                nc.gpsimd.tensor_tensor(out=u3[:, r0:r1, :], in0=w13[:, r0:r1, :],
                                        in1=xp3[:, 1 + r0:1 + r1, 1:129], op=Alu.add)
                nc.vector.tensor_scalar(q1[:, sl], u[:, sl],
                                        pv[:, 4:5], None, Alu.mult)
            for (r0, r1) in HCH:
                sl = slice(128 * r0, 128 * r1)
                nc.vector.tensor_tensor(out=M1[:, sl], in0=u[:, sl],
                                        in1=q1[:, sl], op=Alu.max)

            # ---- T2 = -deadzone_sign(M1 + b13 + b21), dense bf16 ----------
            # deadzone: |t| < theta -> 0 (theta ~ fp32 noise floor), so the
            # reference's exact-zero sign inputs stay zero. Negation is folded
            # into the sign of alpha2 on the host.
            # a = (M1 >= theta - sb2); b = (M1 <= -theta - sb2); the
            # subtraction T2 = a - b is absorbed into conv2 (two accumulating
            # matmuls: +W2^T a then -W2^T b)
            aT = pool.tile([128, NF], bf16, name="aT", tag="F")
            T2 = pool.tile([128, NF], bf16, name="T2", tag="B")
            for (r0, r1) in HCH:
                sl = slice(128 * r0, 128 * r1)
                nc.vector.tensor_scalar(aT[:, sl], M1[:, sl],
                                        pv[:, 15:16], None, Alu.is_ge)
                nc.vector.tensor_scalar(T2[:, sl], M1[:, sl],
                                        pv[:, 16:17], None, Alu.is_le)

            # ---- conv2 (1x1) ----------------------------------------------
            c2p = pool.tile([128, NF], f32, name="c2p", tag="C")
            s2cols = pool.tile([128, 16], f32, name="s2cols")
            pk2 = pool.tile([128, 2], f32, name="pk2")
            s2sq = pool.tile([128, 16], f32, name="s2sq")
            for bg in range(4):
                pss2 = [pp.tile([128, 512], f32, name=f"ps2{bg}_{j}", tag=f"ps{j}")
                        for j in range(4)]
                for t in range(2):
                    for j in range(4):
                        q = 4 * bg + j
                        src = aT if t == 0 else T2
                        nc.tensor.matmul(pss2[j][:, :],
                                         lhsT=w2s[:, 128 * t:128 * (t + 1)],
                                         rhs=src[:, 512 * q:512 * (q + 1)],
                                         start=(t == 0), stop=(t == 1),
                                         skip_group_check=True)
                for j in range(4):
                    q = 4 * bg + j
                    nc.scalar.activation(out=c2p[:, 512 * q:512 * (q + 1)],
                                         in_=pss2[j][:, :], func=Act.Copy,
                                         bias=0.0, scale=1.0,
                                         accum_out=s2cols[:, q:q + 1])
                    nc.scalar.activation(out=sqs[:, 512 * (q % 2):512 * (q % 2 + 1)],
                                         in_=pss2[j][:, :], func=Act.Square,
                                         accum_out=s2sq[:, q:q + 1])
            nc.vector.tensor_reduce(out=pk2[:, 1:2], in_=s2sq[:, :],
                                    axis=mybir.AxisListType.X, op=Alu.add)
            nc.vector.tensor_reduce(out=pk2[:, 0:1], in_=s2cols[:, :],
                                    axis=mybir.AxisListType.X, op=Alu.add)

            # ---- AllReduce branch-2 stats ---------------------------------
            cc2i = dp.tile([128, 2], f32, name="cc2i")
            cc2o = dp.tile([128, 2], f32, name="cc2o", addr_space="Shared")
            nc.gpsimd.dma_start(out=cc2i[:, :], in_=pk2[:, :])
            nc.gpsimd.collective_compute("AllReduce", Alu.add, replica_groups=RG,
                                         ins=[cc2i[:, :].opt()],
                                         outs=[cc2o[:, :].opt()])
            ar2 = pool.tile([128, 4], f32, name="ar2")
            cc2r = cc2o[:, :].rearrange("(h c) s -> c h s", h=2)
            nc.sync.dma_start(out=ar2[0:64, :].rearrange("c (h s) -> c h s", h=2),
                              in_=cc2r)
            nc.sync.dma_start(out=ar2[64:128, :].rearrange("c (h s) -> c h s", h=2),
                              in_=cc2r)

            sm2 = pool.tile([128, 24], f32, name="sm2")
            # pv col6 = gamma2, col7 = beta2+b22+b13, col8 = p2, col10 = b23
            _post_bn_math(nc, sm2, ar2, pv[:, 6:7], pv[:, 7:8], pv[:, 13:14], pv[:, 14:15])

            # ---- branch-2 apply: u2 = (k2-muk2)*G2 + beta2eff + M1;
            #      out = max(u2+b23, p2*u2+b23) = prelu(u2)+b23 -------------
            w2b = pool.tile([128, NF], f32, name="w2b", tag="D")
            u2 = pool.tile([128, NF], f32, name="u2", tag="E")
            q2 = pool.tile([128, NF], f32, name="q2", tag="C")
            outv = pool.tile([128, NF], f32, name="outv", tag="D")
            o3 = outv.rearrange("p (r c) -> p r c", c=128)
            for (r0, r1) in HCH:
                sl = slice(128 * r0, 128 * r1)
                nc.vector.tensor_scalar(w2b[:, sl], c2p[:, sl],
                                        sm2[:, 20:21], sm2[:, 19:20],
                                        Alu.subtract, Alu.mult)
                nc.gpsimd.tensor_tensor(out=u2[:, sl], in0=w2b[:, sl],
                                        in1=M1[:, sl], op=Alu.add)
                nc.vector.tensor_scalar(q2[:, sl], u2[:, sl],
                                        pv[:, 8:9], pv[:, 17:18],
                                        Alu.mult, Alu.add)
            for (r0, r1) in HCH:
                sl = slice(128 * r0, 128 * r1)
                nc.vector.tensor_tensor(out=outv[:, sl], in0=u2[:, sl],
                                        in1=q2[:, sl], op=Alu.max)

            # ---- store ----------------------------------------------------
            for (r0, r1) in RCH:
                nc.sync.dma_start(out=out_t[:, r0:r1, :], in_=o3[0:64, r0:r1, :])
                nc.sync.dma_start(out=out_t[:, 64 + r0:64 + r1, :],
                                  in_=o3[64:128, r0:r1, :])
    return nc


@functools.lru_cache(maxsize=1)
def get_nc():
    nc = build_nc()
    nc.finalize()   # run_bass_kernel_spmd/bass2jax expects a finalized program
    return nc


def _bf16(a):
    import ml_dtypes
    return a.astype(ml_dtypes.bfloat16)


def host_prep(inputs):
    """Build the small derived device inputs from the full problem inputs."""
    w1 = np.asarray(inputs["w1"], np.float32)     # (64, 16, 3, 3)
    w2 = np.asarray(inputs["w2"], np.float32)     # (64, 64, 1, 1)

    a1 = np.mean(np.abs(w1), axis=(1, 2, 3)).astype(np.float32)      # (64,)
    bw1 = np.asarray(_bf16(np.sign(w1)))          # pure +-1, exact in bf16
    a2 = np.mean(np.abs(w2), axis=(1, 2, 3)).astype(np.float32)
    bw2 = np.asarray(_bf16(np.sign(w2)))

    # conv1 lhsT: [K=128 (in-ch x half), 9 taps, M=128 (out-ch x half)]
    w1t = np.zeros((128, 9, 128), dtype=bw1.dtype)
    for o in range(C):
        g = o // 16
        for ir in range(16):
            i_abs = g * 16 + ir
            for t, (dy, dx) in enumerate(TAPS):
                val = bw1[o, ir, dy, dx]
                w1t[i_abs, t, o] = val
                w1t[64 + i_abs, t, 64 + o] = val
    w1t = w1t.reshape(128, 9 * 128)

    w2t = np.zeros((128, 256), dtype=bw2.dtype)
    w2t[0:64, 0:64] = bw2[:, :, 0, 0].T
    w2t[64:128, 64:128] = bw2[:, :, 0, 0].T
    w2t[:, 128:256] = -w2t[:, 0:128]

    def vec(name):
        return np.asarray(inputs[name], np.float32).reshape(C)

    cols = np.zeros((C, 20), np.float32)
    cols[:, 0] = vec("b11")
    cols[:, 1] = vec("b13") + vec("b21")
    cols[:, 2] = vec("bn1_gamma")
    cols[:, 3] = vec("bn1_beta") + vec("b12")
    cols[:, 4] = vec("p1")
    cols[:, 5] = 1.0 - vec("p1")
    cols[:, 6] = vec("bn2_gamma")
    cols[:, 7] = vec("bn2_beta") + vec("b22") + vec("b13") + vec("b23")
    cols[:, 8] = vec("p2")
    cols[:, 9] = 1.0 - vec("p2")
    cols[:, 10] = vec("b23")
    cols[:, 11] = a1
    cols[:, 12] = a1 * a1
    cols[:, 13] = a2
    cols[:, 14] = a2 * a2
    theta = 1e-6
    cols[:, 15] = theta - cols[:, 1]    # M1 >= theta - sb2  <=>  t >= theta
    cols[:, 16] = -theta - cols[:, 1]   # M1 <= -theta - sb2 <=>  t <= -theta
    cols[:, 17] = (1.0 - vec("p2")) * vec("b23")
    pvec = np.concatenate([cols, cols], axis=0)   # dup across halves -> [128,12]
    return w1t, w2t, pvec


def kernel(**inputs):
    from concourse.bass_utils import run_bass_kernel_spmd

    x = np.ascontiguousarray(np.asarray(inputs["x"], np.float32))   # (8,64,128,128)
    loss = np.asarray(inputs["loss"], np.float32)
    w1t, w2t, pvec = host_prep(inputs)

    nc = get_nc()
    in_maps = [{"x": np.ascontiguousarray(x[i]), "w1t": w1t, "w2t": w2t,
                "pvec": pvec} for i in range(NCORES)]
    res = run_bass_kernel_spmd(nc, in_maps, core_ids=list(range(NCORES)))
    out = np.stack([np.asarray(res.results[i]["out"]) for i in range(NCORES)])
    return out, loss


# revision 15
# speedup vs baseline: 1.0946x; 1.0946x over previous
"""Trainium2 Bass kernel for nn_BasicBlock (ReActNet-style binary basic block).

Strategy: data-parallel over batch (8 images -> 8 NeuronCores). All compute for
one image is local except the BatchNorm batch statistics, which are reduced
across cores with two tiny (128x2 fp32) AllReduces inside a single kernel launch.

Per-core layout ("folded"): the 64-channel image is split into two height-halves
stacked on the 128 SBUF partitions (partition c = channel c rows 0..63,
partition 64+c = channel c rows 64..127). The 3x3 conv input is stored padded as
66 rows x 130 cols per partition (1-pixel halo); both halves share identical
access-pattern offsets, so one matmul stream computes both halves using
block-diagonal (duplicated) weights.

Binary ops are exact in bf16: activations are sign() in {-1,0,+1}; weights are
alpha*sign(w) with per-output-channel alpha. bf16 rounding of alpha is a pure
per-channel scale, which BatchNorm divides right back out, so conv in bf16 with
fp32 PSUM accumulation reproduces the fp32 reference exactly.

Bias/PReLU algebra is folded into per-channel scalars so the whole post-conv
chain is 4 fused passes:
  u = (c * g') + residual          [DVE scalar_tensor_tensor]
  r = Relu(u + Bp)                 [ACT, per-partition bias]
  q = (u * p) + (p*Bp [+ b23])     [DVE tensor_scalar dual-op]
  M = (r * (1-p)) + q              [GPSIMD scalar_tensor_tensor]
which equals prelu(u + Bp) [+ b23] since prelu(z) = (1-p)*relu(z) + p*z.
"""

import functools
import numpy as np

import concourse.bass as bass
import concourse.bacc as bacc
import concourse.tile as tile
from concourse import mybir

EPS = 1e-5
B, C, H = 8, 64, 128
NCORES = 8
PAD = 130          # padded row length (128 + 2)
ROWS = 66          # padded rows per half (64 + 2 halo)
XF = ROWS * PAD    # 8580 padded elements per partition
NF = 64 * 128      # 8192 dense elements per partition (half image)
NTOT = B * H * H   # 131072: count per channel for batch stats
TAPS = [(dy, dx) for dy in range(3) for dx in range(3)]

f32 = mybir.dt.float32
bf16 = mybir.dt.bfloat16
Alu = mybir.AluOpType
Act = mybir.ActivationFunctionType


def _post_bn_math(nc, sm, ar, gamma_col, beta_col, alpha_col, alphasq_col):
    """Tiny per-partition ([128,1]) math turning AllReduced integer count
    sums into per-channel scalars. The conv ran with pure sign(+-1) weights,
    so sums are exact integer counts; alpha is applied here in fp32.

    Writes sm cols: 2 = muk (mean in count units), 19 = G (gamma*rsqrt*alpha).
    The apply pass computes (k - muk)*G == gamma*(c - mu)*rsqrt(var+eps),
    matching the reference's operation order.
    """
    TT = nc.vector.tensor_tensor
    TS = nc.vector.tensor_scalar

    def c(i):
        return sm[:, i:i + 1]

    TT(out=c(0), in0=ar[:, 0:1], in1=ar[:, 2:3], op=Alu.add)      # S1 counts
    TT(out=c(1), in0=ar[:, 1:2], in1=ar[:, 3:4], op=Alu.add)      # S2 counts
    TS(c(2), c(0), 1.0 / NTOT, None, Alu.mult)                    # muk
    TS(c(3), c(1), 1.0 / NTOT, None, Alu.mult)                    # E[k^2]
    TS(c(4), c(2), c(2), None, Alu.mult)                          # muk^2
    TT(out=c(5), in0=c(3), in1=c(4), op=Alu.subtract)             # vark
    TT(out=c(6), in0=c(5), in1=alphasq_col, op=Alu.mult)          # var
    TS(c(6), c(6), EPS, None, Alu.add)                            # var+eps
    nc.scalar.activation(out=c(7), in_=c(6), func=Act.Sqrt)       # ~sqrt
    nc.vector.reciprocal(out=c(8), in_=c(7))                      # y0 ~ rsqrt
    # one Newton step: y = y0*(1.5 - 0.5*(var+eps)*y0^2)
    TS(c(9), c(8), c(8), None, Alu.mult)                          # y0^2
    TT(out=c(10), in0=c(9), in1=c(6), op=Alu.mult)                # v*y0^2
    TS(c(11), c(10), -0.5, 1.5, Alu.mult, Alu.add)                # 1.5-0.5*
    TT(out=c(12), in0=c(8), in1=c(11), op=Alu.mult)               # y (rsqrt)
    TT(out=c(13), in0=c(12), in1=gamma_col, op=Alu.mult)          # g'
    TT(out=c(19), in0=c(13), in1=alpha_col, op=Alu.mult)          # G = g'*alpha
    # fold the additive bias beta into the subtracted mean so the residual
    # add needs no per-channel scalar: (k - (muk - beta/G))*G = (k-muk)*G+beta
    nc.vector.reciprocal(out=c(14), in_=c(19))                    # 1/G
    TT(out=c(15), in0=beta_col, in1=c(14), op=Alu.mult)           # beta/G
    TT(out=c(20), in0=c(2), in1=c(15), op=Alu.subtract)           # muk'


def build_nc():
    nc = bacc.Bacc(None, num_devices=NCORES)

    x_t = nc.declare_dram_parameter("x", [C, H, H], f32, isOutput=False)
    w1_t = nc.declare_dram_parameter("w1t", [128, 9 * 128], bf16, isOutput=False)
    w2_t = nc.declare_dram_parameter("w2t", [128, 256], bf16, isOutput=False)
    pv_t = nc.declare_dram_parameter("pvec", [128, 20], f32, isOutput=False)
    out_t = nc.declare_dram_parameter("out", [C, H, H], f32, isOutput=True)

    RG = [list(range(NCORES))]
    # row-chunk boundaries for chunked full-image passes
    RCH = [(0, 16), (16, 32), (32, 48), (48, 64)]
    HCH = [(0, 32), (32, 64)]

    with tile.TileContext(nc, num_cores=NCORES) as tc:
        with tc.tile_pool(name="main", bufs=1) as pool, \
             tc.tile_pool(name="pp", bufs=2, space="PSUM") as pp, \
             tc.tile_pool(name="dp", bufs=1, space="DRAM") as dp:

            w1s = pool.tile([128, 9 * 128], bf16, name="w1s")
            w2s = pool.tile([128, 256], bf16, name="w2s")
            pv = pool.tile([128, 20], f32, name="pv")
            nc.sync.dma_start(out=w1s[:, :], in_=w1_t[:, :])
            nc.sync.dma_start(out=w2s[:, :], in_=w2_t[:, :])
            nc.sync.dma_start(out=pv[:, :], in_=pv_t[:, :])
            # preload the sqrt ACT table set now, not in the middle of the
            # AllReduce latency window
            warmsq = pool.tile([128, 1], f32, name="warmsq")
            nc.scalar.activation(out=warmsq[:, :], in_=pv[:, 12:13], func=Act.Sqrt)

            # ---- load x into padded/folded layout -------------------------
            xpad = pool.tile([128, XF], f32, name="xpad", tag="A")
            xp3 = xpad.rearrange("p (r c) -> p r c", c=PAD)
            # zero borders: top halo row of half0, bottom halo row of half1,
            # left and right halo columns
            nc.vector.memset(xp3[0:64, 0:1, :], 0.0)
            nc.vector.memset(xp3[64:128, 65:66, :], 0.0)
            nc.vector.memset(xp3[:, :, 0:1], 0.0)
            nc.vector.memset(xp3[:, :, 129:130], 0.0)
            for (r0, r1) in [(0, 17), (17, 33), (33, 49), (49, 65)]:
                # half0 partitions: padded row r holds image row r-1 -> rows
                # 1..65 hold image rows 0..64 (incl. 1-row halo below)
                nc.sync.dma_start(out=xp3[0:64, 1 + r0:1 + r1, 1:129],
                                  in_=x_t[:, r0:r1, :])
                # half1 partitions: padded row r holds image row 63+r
                nc.sync.dma_start(out=xp3[64:128, r0:r1, 1:129],
                                  in_=x_t[:, 63 + r0:63 + r1, :])

            # ---- T1 = sign(x + b11), padded, bf16 -------------------------
            T1 = pool.tile([128, XF], bf16, name="T1", tag="B")
            T13 = T1.rearrange("p (r c) -> p r c", c=PAD)
            wps = pp.tile([128, 512], f32, name="wps", tag="ps0")
            for (r0, r1) in [(0, 17), (17, 33), (33, 50), (50, 66)]:
                nc.scalar.activation(out=T13[:, r0:r1, :], in_=xp3[:, r0:r1, :],
                                     func=Act.Sign, bias=pv[:, 0:1])
                # HAM warmup: keep the PE busy during the load/sign phase so
                # conv1 runs at 2.4 GHz from its first matmul
                for _ in range(8):
                    nc.tensor.matmul(wps[:, :], lhsT=w1s[:, 0:128],
                                     rhs=T13[:, r0:r0 + 4, 0:128],
                                     start=True, stop=True,
                                     skip_group_check=True)
            # restore zero borders (sign(0+b11) may be nonzero there)
            nc.vector.memset(T13[0:64, 0:1, :], 0.0)
            nc.vector.memset(T13[64:128, 65:66, :], 0.0)
            nc.vector.memset(T13[:, :, 0:1], 0.0)
            nc.vector.memset(T13[:, :, 129:130], 0.0)

            # ---- conv1: 9 accumulating matmuls per 512-col chunk ----------
            c1p = pool.tile([128, NF], f32, name="c1p", tag="C")
            s1cols = pool.tile([128, 8], f32, name="s1cols")
            pk1 = pool.tile([128, 2], f32, name="pk1")
            s1sq = pool.tile([128, 8], f32, name="s1sq")
            sqs = pool.tile([128, 2 * 1024], bf16, name="sqs")
            # 8 double-chunks of 1024 cols; each psum tile spans 2 banks and
            # is drained by ONE copy + ONE square (halves ACT op count)
            for bg in range(4):
                pss = [pp.tile([128, 1024], f32, name=f"ps{bg}_{j}", tag=f"ps{j}")
                       for j in range(2)]
                # taps outer: the stationary operand is reused across the
                # group, so LDWEIGHTS amortizes and the PE gets dense
                # back-to-back matmul bursts (keeps HAM un-throttled)
                for t, (dy, dx) in enumerate(TAPS):
                    for j in range(2):
                        for h in range(2):
                            q = 4 * bg + 2 * j + h
                            nc.tensor.matmul(
                                pss[j][:, 512 * h:512 * (h + 1)],
                                lhsT=w1s[:, t * 128:(t + 1) * 128],
                                rhs=T13[:, 4 * q + dy:4 * q + dy + 4, dx:dx + 128],
                                start=(t == 0), stop=(t == 8),
                                skip_group_check=True)
                for j in range(2):
                    dq = 2 * bg + j
                    nc.scalar.activation(out=c1p[:, 1024 * dq:1024 * (dq + 1)],
                                         in_=pss[j][:, :], func=Act.Copy,
                                         bias=0.0, scale=1.0,
                                         accum_out=s1cols[:, dq:dq + 1])
                    nc.scalar.activation(out=sqs[:, 1024 * j:1024 * (j + 1)],
                                         in_=pss[j][:, :], func=Act.Square,
                                         accum_out=s1sq[:, dq:dq + 1])

            nc.vector.tensor_reduce(out=pk1[:, 1:2], in_=s1sq[:, :],
                                    axis=mybir.AxisListType.X, op=Alu.add)
            nc.vector.tensor_reduce(out=pk1[:, 0:1], in_=s1cols[:, :],
                                    axis=mybir.AxisListType.X, op=Alu.add)

            # ---- AllReduce branch-1 stats ---------------------------------
            cc1i = dp.tile([128, 2], f32, name="cc1i")
            cc1o = dp.tile([128, 2], f32, name="cc1o", addr_space="Shared")
            nc.gpsimd.dma_start(out=cc1i[:, :], in_=pk1[:, :])
            nc.gpsimd.collective_compute("AllReduce", Alu.add, replica_groups=RG,
                                         ins=[cc1i[:, :].opt()],
                                         outs=[cc1o[:, :].opt()])
            ar1 = pool.tile([128, 4], f32, name="ar1")
            cc1r = cc1o[:, :].rearrange("(h c) s -> c h s", h=2)
            nc.sync.dma_start(out=ar1[0:64, :].rearrange("c (h s) -> c h s", h=2),
                              in_=cc1r)
            nc.sync.dma_start(out=ar1[64:128, :].rearrange("c (h s) -> c h s", h=2),
                              in_=cc1r)

            sm1 = pool.tile([128, 24], f32, name="sm1")
            # pv col2 = gamma1, col3 = beta1+b12, col4 = p1
            _post_bn_math(nc, sm1, ar1, pv[:, 2:3], pv[:, 3:4], pv[:, 11:12], pv[:, 12:13])

            # ---- branch-1 apply: bn = (k-muk)*G; u = bn+beta12+x;
            #      M1 = prelu(u) = max(u, p1*u) ----------------------------
            w1b = pool.tile([128, NF], f32, name="w1b", tag="D")
            w13 = w1b.rearrange("p (r c) -> p r c", c=128)
            u = pool.tile([128, NF], f32, name="u", tag="E")
            u3 = u.rearrange("p (r c) -> p r c", c=128)
            q1 = pool.tile([128, NF], f32, name="q1", tag="C")
            M1 = pool.tile([128, NF], f32, name="M1", tag="A")
            M13 = M1.rearrange("p (r c) -> p r c", c=128)
            for (r0, r1) in HCH:
                sl = slice(128 * r0, 128 * r1)
                nc.vector.tensor_scalar(w1b[:, sl], c1p[:, sl],
                                        sm1[:, 20:21], sm1[:, 19:20],
                                        Alu.subtract, Alu.mult)
                nc.vector.tensor_tensor(out=u3[:, r0:r1, :], in0=w13[:, r0:r1, :],
                                        in1=xp3[:, 1 + r0:1 + r1, 1:129], op=Alu.add)
                nc.vector.tensor_scalar(q1[:, sl], u[:, sl],
                                        pv[:, 4:5], None, Alu.mult)
            for (r0, r1) in HCH:
                sl = slice(128 * r0, 128 * r1)
                nc.vector.tensor_tensor(out=M1[:, sl], in0=u[:, sl],
                                        in1=q1[:, sl], op=Alu.max)

            # ---- T2 = -deadzone_sign(M1 + b13 + b21), dense bf16 ----------
            # deadzone: |t| < theta -> 0 (theta ~ fp32 noise floor), so the
            # reference's exact-zero sign inputs stay zero. Negation is folded
            # into the sign of alpha2 on the host.
            # a = (M1 >= theta - sb2); b = (M1 <= -theta - sb2); the
            # subtraction T2 = a - b is absorbed into conv2 (two accumulating
            # matmuls: +W2^T a then -W2^T b)
            aT = pool.tile([128, NF], bf16, name="aT", tag="F")
            T2 = pool.tile([128, NF], bf16, name="T2", tag="B")
            for (r0, r1) in HCH:
                sl = slice(128 * r0, 128 * r1)
                nc.vector.tensor_scalar(aT[:, sl], M1[:, sl],
                                        pv[:, 15:16], None, Alu.is_ge)
                nc.vector.tensor_scalar(T2[:, sl], M1[:, sl],
                                        pv[:, 16:17], None, Alu.is_le)

            # ---- conv2 (1x1) ----------------------------------------------
            c2p = pool.tile([128, NF], f32, name="c2p", tag="C")
            s2cols = pool.tile([128, 8], f32, name="s2cols")
            pk2 = pool.tile([128, 2], f32, name="pk2")
            s2sq = pool.tile([128, 8], f32, name="s2sq")
            for bg in range(4):
                pss2 = [pp.tile([128, 1024], f32, name=f"ps2{bg}_{j}", tag=f"ps{j}")
                        for j in range(2)]
                for t in range(2):
                    for j in range(2):
                        for h in range(2):
                            q = 4 * bg + 2 * j + h
                            src = aT if t == 0 else T2
                            nc.tensor.matmul(
                                pss2[j][:, 512 * h:512 * (h + 1)],
                                lhsT=w2s[:, 128 * t:128 * (t + 1)],
                                rhs=src[:, 512 * q:512 * (q + 1)],
                                start=(t == 0), stop=(t == 1),
                                skip_group_check=True)
                for j in range(2):
                    dq = 2 * bg + j
                    nc.scalar.activation(out=c2p[:, 1024 * dq:1024 * (dq + 1)],
                                         in_=pss2[j][:, :], func=Act.Copy,
                                         bias=0.0, scale=1.0,
                                         accum_out=s2cols[:, dq:dq + 1])
                    nc.scalar.activation(out=sqs[:, 1024 * j:1024 * (j + 1)],
                                         in_=pss2[j][:, :], func=Act.Square,
                                         accum_out=s2sq[:, dq:dq + 1])
            nc.vector.tensor_reduce(out=pk2[:, 1:2], in_=s2sq[:, :],
                                    axis=mybir.AxisListType.X, op=Alu.add)
            nc.vector.tensor_reduce(out=pk2[:, 0:1], in_=s2cols[:, :],
                                    axis=mybir.AxisListType.X, op=Alu.add)

            # ---- AllReduce branch-2 stats ---------------------------------
            cc2i = dp.tile([128, 2], f32, name="cc2i")
            cc2o = dp.tile([128, 2], f32, name="cc2o", addr_space="Shared")
            nc.gpsimd.dma_start(out=cc2i[:, :], in_=pk2[:, :])
            nc.gpsimd.collective_compute("AllReduce", Alu.add, replica_groups=RG,
                                         ins=[cc2i[:, :].opt()],
                                         outs=[cc2o[:, :].opt()])
            ar2 = pool.tile([128, 4], f32, name="ar2")
            cc2r = cc2o[:, :].rearrange("(h c) s -> c h s", h=2)
            nc.sync.dma_start(out=ar2[0:64, :].rearrange("c (h s) -> c h s", h=2),
                              in_=cc2r)
            nc.sync.dma_start(out=ar2[64:128, :].rearrange("c (h s) -> c h s", h=2),
                              in_=cc2r)

            sm2 = pool.tile([128, 24], f32, name="sm2")
            # pv col6 = gamma2, col7 = beta2+b22+b13, col8 = p2, col10 = b23
            _post_bn_math(nc, sm2, ar2, pv[:, 6:7], pv[:, 7:8], pv[:, 13:14], pv[:, 14:15])

            # ---- branch-2 apply: u2 = (k2-muk2)*G2 + beta2eff + M1;
            #      out = max(u2+b23, p2*u2+b23) = prelu(u2)+b23 -------------
            w2b = pool.tile([128, NF], f32, name="w2b", tag="D")
            u2 = pool.tile([128, NF], f32, name="u2", tag="E")
            q2 = pool.tile([128, NF], f32, name="q2", tag="C")
            outv = pool.tile([128, NF], f32, name="outv", tag="D")
            o3 = outv.rearrange("p (r c) -> p r c", c=128)
            for (r0, r1) in HCH:
                sl = slice(128 * r0, 128 * r1)
                nc.vector.tensor_scalar(w2b[:, sl], c2p[:, sl],
                                        sm2[:, 20:21], sm2[:, 19:20],
                                        Alu.subtract, Alu.mult)
                nc.vector.tensor_tensor(out=u2[:, sl], in0=w2b[:, sl],
                                        in1=M1[:, sl], op=Alu.add)
                nc.vector.tensor_scalar(q2[:, sl], u2[:, sl],
                                        pv[:, 8:9], pv[:, 17:18],
                                        Alu.mult, Alu.add)
            for (r0, r1) in HCH:
                sl = slice(128 * r0, 128 * r1)
                nc.vector.tensor_tensor(out=outv[:, sl], in0=u2[:, sl],
                                        in1=q2[:, sl], op=Alu.max)

            # ---- store ----------------------------------------------------
            for (r0, r1) in RCH:
                nc.sync.dma_start(out=out_t[:, r0:r1, :], in_=o3[0:64, r0:r1, :])
                nc.sync.dma_start(out=out_t[:, 64 + r0:64 + r1, :],
                                  in_=o3[64:128, r0:r1, :])
    return nc


@functools.lru_cache(maxsize=1)
def get_nc():
    nc = build_nc()
    nc.finalize()   # run_bass_kernel_spmd/bass2jax expects a finalized program
    return nc


def _bf16(a):
    import ml_dtypes
    return a.astype(ml_dtypes.bfloat16)


def host_prep(inputs):
    """Build the small derived device inputs from the full problem inputs."""
    w1 = np.asarray(inputs["w1"], np.float32)     # (64, 16, 3, 3)
    w2 = np.asarray(inputs["w2"], np.float32)     # (64, 64, 1, 1)

    a1 = np.mean(np.abs(w1), axis=(1, 2, 3)).astype(np.float32)      # (64,)
    bw1 = np.asarray(_bf16(np.sign(w1)))          # pure +-1, exact in bf16
    a2 = np.mean(np.abs(w2), axis=(1, 2, 3)).astype(np.float32)
    bw2 = np.asarray(_bf16(np.sign(w2)))

    # conv1 lhsT: [K=128 (in-ch x half), 9 taps, M=128 (out-ch x half)]
    w1t = np.zeros((128, 9, 128), dtype=bw1.dtype)
    for o in range(C):
        g = o // 16
        for ir in range(16):
            i_abs = g * 16 + ir
            for t, (dy, dx) in enumerate(TAPS):
                val = bw1[o, ir, dy, dx]
                w1t[i_abs, t, o] = val
                w1t[64 + i_abs, t, 64 + o] = val
    w1t = w1t.reshape(128, 9 * 128)

    w2t = np.zeros((128, 256), dtype=bw2.dtype)
    w2t[0:64, 0:64] = bw2[:, :, 0, 0].T
    w2t[64:128, 64:128] = bw2[:, :, 0, 0].T
    w2t[:, 128:256] = -w2t[:, 0:128]

    def vec(name):
        return np.asarray(inputs[name], np.float32).reshape(C)

    cols = np.zeros((C, 20), np.float32)
    cols[:, 0] = vec("b11")
    cols[:, 1] = vec("b13") + vec("b21")
    cols[:, 2] = vec("bn1_gamma")
    cols[:, 3] = vec("bn1_beta") + vec("b12")
    cols[:, 4] = vec("p1")
    cols[:, 5] = 1.0 - vec("p1")
    cols[:, 6] = vec("bn2_gamma")
    cols[:, 7] = vec("bn2_beta") + vec("b22") + vec("b13") + vec("b23")
    cols[:, 8] = vec("p2")
    cols[:, 9] = 1.0 - vec("p2")
    cols[:, 10] = vec("b23")
    cols[:, 11] = a1
    cols[:, 12] = a1 * a1
    cols[:, 13] = a2
    cols[:, 14] = a2 * a2
    theta = 1e-6
    cols[:, 15] = theta - cols[:, 1]    # M1 >= theta - sb2  <=>  t >= theta
    cols[:, 16] = -theta - cols[:, 1]   # M1 <= -theta - sb2 <=>  t <= -theta
    cols[:, 17] = (1.0 - vec("p2")) * vec("b23")
    pvec = np.concatenate([cols, cols], axis=0)   # dup across halves -> [128,12]
    return w1t, w2t, pvec


def kernel(**inputs):
    from concourse.bass_utils import run_bass_kernel_spmd

    x = np.ascontiguousarray(np.asarray(inputs["x"], np.float32))   # (8,64,128,128)
    loss = np.asarray(inputs["loss"], np.float32)
    w1t, w2t, pvec = host_prep(inputs)

    nc = get_nc()
    in_maps = [{"x": np.ascontiguousarray(x[i]), "w1t": w1t, "w2t": w2t,
                "pvec": pvec} for i in range(NCORES)]
    res = run_bass_kernel_spmd(nc, in_maps, core_ids=list(range(NCORES)))
    out = np.stack([np.asarray(res.results[i]["out"]) for i in range(NCORES)])
    return out, loss


# revision 16
# speedup vs baseline: 1.0960x; 1.0012x over previous
"""Trainium2 Bass kernel for nn_BasicBlock (ReActNet-style binary basic block).

Strategy: data-parallel over batch (8 images -> 8 NeuronCores). All compute for
one image is local except the BatchNorm batch statistics, which are reduced
across cores with two tiny (128x2 fp32) AllReduces inside a single kernel launch.

Per-core layout ("folded"): the 64-channel image is split into two height-halves
stacked on the 128 SBUF partitions (partition c = channel c rows 0..63,
partition 64+c = channel c rows 64..127). The 3x3 conv input is stored padded as
66 rows x 130 cols per partition (1-pixel halo); both halves share identical
access-pattern offsets, so one matmul stream computes both halves using
block-diagonal (duplicated) weights.

Binary ops are exact in bf16: activations are sign() in {-1,0,+1}; weights are
alpha*sign(w) with per-output-channel alpha. bf16 rounding of alpha is a pure
per-channel scale, which BatchNorm divides right back out, so conv in bf16 with
fp32 PSUM accumulation reproduces the fp32 reference exactly.

Bias/PReLU algebra is folded into per-channel scalars so the whole post-conv
chain is 4 fused passes:
  u = (c * g') + residual          [DVE scalar_tensor_tensor]
  r = Relu(u + Bp)                 [ACT, per-partition bias]
  q = (u * p) + (p*Bp [+ b23])     [DVE tensor_scalar dual-op]
  M = (r * (1-p)) + q              [GPSIMD scalar_tensor_tensor]
which equals prelu(u + Bp) [+ b23] since prelu(z) = (1-p)*relu(z) + p*z.
"""

import functools
import numpy as np

import concourse.bass as bass
import concourse.bacc as bacc
import concourse.tile as tile
from concourse import mybir

EPS = 1e-5
B, C, H = 8, 64, 128
NCORES = 8
PAD = 130          # padded row length (128 + 2)
ROWS = 66          # padded rows per half (64 + 2 halo)
XF = ROWS * PAD    # 8580 padded elements per partition
NF = 64 * 128      # 8192 dense elements per partition (half image)
NTOT = B * H * H   # 131072: count per channel for batch stats
TAPS = [(dy, dx) for dy in range(3) for dx in range(3)]

f32 = mybir.dt.float32
bf16 = mybir.dt.bfloat16
Alu = mybir.AluOpType
Act = mybir.ActivationFunctionType


def _post_bn_math(nc, sm, ar, gamma_col, beta_col, alpha_col, alphasq_col):
    """Tiny per-partition ([128,1]) math turning AllReduced integer count
    sums into per-channel scalars. The conv ran with pure sign(+-1) weights,
    so sums are exact integer counts; alpha is applied here in fp32.

    Writes sm cols: 2 = muk (mean in count units), 19 = G (gamma*rsqrt*alpha).
    The apply pass computes (k - muk)*G == gamma*(c - mu)*rsqrt(var+eps),
    matching the reference's operation order.
    """
    TT = nc.vector.tensor_tensor
    TS = nc.vector.tensor_scalar

    def c(i):
        return sm[:, i:i + 1]

    TT(out=c(0), in0=ar[:, 0:1], in1=ar[:, 2:3], op=Alu.add)      # S1 counts
    TT(out=c(1), in0=ar[:, 1:2], in1=ar[:, 3:4], op=Alu.add)      # S2 counts
    TS(c(2), c(0), 1.0 / NTOT, None, Alu.mult)                    # muk
    TS(c(3), c(1), 1.0 / NTOT, None, Alu.mult)                    # E[k^2]
    TS(c(4), c(2), c(2), None, Alu.mult)                          # muk^2
    TT(out=c(5), in0=c(3), in1=c(4), op=Alu.subtract)             # vark
    TT(out=c(6), in0=c(5), in1=alphasq_col, op=Alu.mult)          # var
    TS(c(6), c(6), EPS, None, Alu.add)                            # var+eps
    nc.scalar.activation(out=c(7), in_=c(6), func=Act.Sqrt)       # ~sqrt
    nc.vector.reciprocal(out=c(8), in_=c(7))                      # y0 ~ rsqrt
    # one Newton step: y = y0*(1.5 - 0.5*(var+eps)*y0^2)
    TS(c(9), c(8), c(8), None, Alu.mult)                          # y0^2
    TT(out=c(10), in0=c(9), in1=c(6), op=Alu.mult)                # v*y0^2
    TS(c(11), c(10), -0.5, 1.5, Alu.mult, Alu.add)                # 1.5-0.5*
    TT(out=c(12), in0=c(8), in1=c(11), op=Alu.mult)               # y (rsqrt)
    TT(out=c(13), in0=c(12), in1=gamma_col, op=Alu.mult)          # g'
    TT(out=c(19), in0=c(13), in1=alpha_col, op=Alu.mult)          # G = g'*alpha
    # fold the additive bias beta into the subtracted mean so the residual
    # add needs no per-channel scalar: (k - (muk - beta/G))*G = (k-muk)*G+beta
    nc.vector.reciprocal(out=c(14), in_=c(19))                    # 1/G
    TT(out=c(15), in0=beta_col, in1=c(14), op=Alu.mult)           # beta/G
    TT(out=c(20), in0=c(2), in1=c(15), op=Alu.subtract)           # muk'


def build_nc():
    nc = bacc.Bacc(None, num_devices=NCORES)

    x_t = nc.declare_dram_parameter("x", [C, H, H], f32, isOutput=False)
    w1_t = nc.declare_dram_parameter("w1t", [128, 9 * 128], bf16, isOutput=False)
    w2_t = nc.declare_dram_parameter("w2t", [128, 256], bf16, isOutput=False)
    pv_t = nc.declare_dram_parameter("pvec", [128, 20], f32, isOutput=False)
    out_t = nc.declare_dram_parameter("out", [C, H, H], f32, isOutput=True)

    RG = [list(range(NCORES))]
    # row-chunk boundaries for chunked full-image passes
    RCH = [(0, 16), (16, 32), (32, 48), (48, 64)]
    HCH = [(0, 32), (32, 64)]

    with tile.TileContext(nc, num_cores=NCORES) as tc:
        with tc.tile_pool(name="main", bufs=1) as pool, \
             tc.tile_pool(name="pp", bufs=2, space="PSUM") as pp, \
             tc.tile_pool(name="dp", bufs=1, space="DRAM") as dp:

            w1s = pool.tile([128, 9 * 128], bf16, name="w1s")
            w2s = pool.tile([128, 256], bf16, name="w2s")
            pv = pool.tile([128, 20], f32, name="pv")
            nc.sync.dma_start(out=w1s[:, :], in_=w1_t[:, :])
            nc.sync.dma_start(out=w2s[:, :], in_=w2_t[:, :])
            nc.sync.dma_start(out=pv[:, :], in_=pv_t[:, :])
            # preload the sqrt ACT table set now, not in the middle of the
            # AllReduce latency window
            warmsq = pool.tile([128, 1], f32, name="warmsq")
            nc.scalar.activation(out=warmsq[:, :], in_=pv[:, 12:13], func=Act.Sqrt)

            # ---- load x into padded/folded layout -------------------------
            xpad = pool.tile([128, XF], f32, name="xpad", tag="A")
            xp3 = xpad.rearrange("p (r c) -> p r c", c=PAD)
            # zero borders: top halo row of half0, bottom halo row of half1,
            # left and right halo columns
            nc.vector.memset(xp3[0:64, 0:1, :], 0.0)
            nc.vector.memset(xp3[64:128, 65:66, :], 0.0)
            nc.vector.memset(xp3[:, :, 0:1], 0.0)
            nc.vector.memset(xp3[:, :, 129:130], 0.0)
            # chunk boundaries aligned with the sign/conv chunks below so the
            # pipeline can start convolving while later rows still stream in
            for (d0, d1) in [(0, 19), (19, 35), (35, 51), (51, 66)]:
                # half0 partitions: padded row r holds image row r-1
                h0, h1 = max(d0, 1), d1
                nc.sync.dma_start(out=xp3[0:64, h0:h1, 1:129],
                                  in_=x_t[:, h0 - 1:h1 - 1, :])
                # half1 partitions: padded row r holds image row 63+r
                g0, g1 = d0, min(d1, 65)
                nc.sync.dma_start(out=xp3[64:128, g0:g1, 1:129],
                                  in_=x_t[:, 63 + g0:63 + g1, :])

            # ---- T1 = sign(x + b11), padded, bf16 -------------------------
            T1 = pool.tile([128, XF], bf16, name="T1", tag="B")
            T13 = T1.rearrange("p (r c) -> p r c", c=PAD)
            wps = pp.tile([128, 512], f32, name="wps", tag="ps0")
            for (r0, r1) in [(0, 19), (19, 35), (35, 51), (51, 66)]:
                nc.scalar.activation(out=T13[:, r0:r1, :], in_=xp3[:, r0:r1, :],
                                     func=Act.Sign, bias=pv[:, 0:1])
                # HAM warmup: keep the PE busy during the load/sign phase so
                # conv1 runs at 2.4 GHz from its first matmul
                for _ in range(8):
                    nc.tensor.matmul(wps[:, :], lhsT=w1s[:, 0:128],
                                     rhs=T13[:, r0:r0 + 4, 0:128],
                                     start=True, stop=True,
                                     skip_group_check=True)
            # restore zero borders (sign(0+b11) may be nonzero there)
            nc.vector.memset(T13[0:64, 0:1, :], 0.0)
            nc.vector.memset(T13[64:128, 65:66, :], 0.0)
            nc.vector.memset(T13[:, :, 0:1], 0.0)
            nc.vector.memset(T13[:, :, 129:130], 0.0)

            # ---- conv1: 9 accumulating matmuls per 512-col chunk ----------
            c1p = pool.tile([128, NF], f32, name="c1p", tag="C")
            s1cols = pool.tile([128, 8], f32, name="s1cols")
            pk1 = pool.tile([128, 2], f32, name="pk1")
            s1sq = pool.tile([128, 8], f32, name="s1sq")
            sqs = pool.tile([128, 2 * 1024], bf16, name="sqs")
            # 8 double-chunks of 1024 cols; each psum tile spans 2 banks and
            # is drained by ONE copy + ONE square (halves ACT op count)
            for bg in range(4):
                pss = [pp.tile([128, 1024], f32, name=f"ps{bg}_{j}", tag=f"ps{j}")
                       for j in range(2)]
                # taps outer: the stationary operand is reused across the
                # group, so LDWEIGHTS amortizes and the PE gets dense
                # back-to-back matmul bursts (keeps HAM un-throttled)
                for t, (dy, dx) in enumerate(TAPS):
                    for j in range(2):
                        for h in range(2):
                            q = 4 * bg + 2 * j + h
                            nc.tensor.matmul(
                                pss[j][:, 512 * h:512 * (h + 1)],
                                lhsT=w1s[:, t * 128:(t + 1) * 128],
                                rhs=T13[:, 4 * q + dy:4 * q + dy + 4, dx:dx + 128],
                                start=(t == 0), stop=(t == 8),
                                skip_group_check=True)
                for j in range(2):
                    dq = 2 * bg + j
                    nc.scalar.activation(out=c1p[:, 1024 * dq:1024 * (dq + 1)],
                                         in_=pss[j][:, :], func=Act.Copy,
                                         bias=0.0, scale=1.0,
                                         accum_out=s1cols[:, dq:dq + 1])
                    nc.scalar.activation(out=sqs[:, 1024 * j:1024 * (j + 1)],
                                         in_=pss[j][:, :], func=Act.Square,
                                         accum_out=s1sq[:, dq:dq + 1])

            nc.vector.tensor_reduce(out=pk1[:, 1:2], in_=s1sq[:, :],
                                    axis=mybir.AxisListType.X, op=Alu.add)
            nc.vector.tensor_reduce(out=pk1[:, 0:1], in_=s1cols[:, :],
                                    axis=mybir.AxisListType.X, op=Alu.add)

            # ---- AllReduce branch-1 stats ---------------------------------
            cc1i = dp.tile([128, 2], f32, name="cc1i")
            cc1o = dp.tile([128, 2], f32, name="cc1o", addr_space="Shared")
            nc.gpsimd.dma_start(out=cc1i[:, :], in_=pk1[:, :])
            nc.gpsimd.collective_compute("AllReduce", Alu.add, replica_groups=RG,
                                         ins=[cc1i[:, :].opt()],
                                         outs=[cc1o[:, :].opt()])
            ar1 = pool.tile([128, 4], f32, name="ar1")
            cc1r = cc1o[:, :].rearrange("(h c) s -> c h s", h=2)
            nc.sync.dma_start(out=ar1[0:64, :].rearrange("c (h s) -> c h s", h=2),
                              in_=cc1r)
            nc.sync.dma_start(out=ar1[64:128, :].rearrange("c (h s) -> c h s", h=2),
                              in_=cc1r)

            sm1 = pool.tile([128, 24], f32, name="sm1")
            # pv col2 = gamma1, col3 = beta1+b12, col4 = p1
            _post_bn_math(nc, sm1, ar1, pv[:, 2:3], pv[:, 3:4], pv[:, 11:12], pv[:, 12:13])

            # ---- branch-1 apply: bn = (k-muk)*G; u = bn+beta12+x;
            #      M1 = prelu(u) = max(u, p1*u) ----------------------------
            w1b = pool.tile([128, NF], f32, name="w1b", tag="D")
            w13 = w1b.rearrange("p (r c) -> p r c", c=128)
            u = pool.tile([128, NF], f32, name="u", tag="E")
            u3 = u.rearrange("p (r c) -> p r c", c=128)
            M1 = pool.tile([128, NF], f32, name="M1", tag="A")
            M13 = M1.rearrange("p (r c) -> p r c", c=128)
            for (r0, r1) in HCH:
                sl = slice(128 * r0, 128 * r1)
                nc.vector.tensor_scalar(w1b[:, sl], c1p[:, sl],
                                        sm1[:, 20:21], sm1[:, 19:20],
                                        Alu.subtract, Alu.mult)
                nc.vector.tensor_tensor(out=u3[:, r0:r1, :], in0=w13[:, r0:r1, :],
                                        in1=xp3[:, 1 + r0:1 + r1, 1:129], op=Alu.add)
            for (r0, r1) in HCH:
                sl = slice(128 * r0, 128 * r1)
                nc.vector.scalar_tensor_tensor(out=M1[:, sl], in0=u[:, sl],
                                               scalar=pv[:, 4:5], in1=u[:, sl],
                                               op0=Alu.mult, op1=Alu.max)

            # ---- T2 = -deadzone_sign(M1 + b13 + b21), dense bf16 ----------
            # deadzone: |t| < theta -> 0 (theta ~ fp32 noise floor), so the
            # reference's exact-zero sign inputs stay zero. Negation is folded
            # into the sign of alpha2 on the host.
            # a = (M1 >= theta - sb2); b = (M1 <= -theta - sb2); the
            # subtraction T2 = a - b is absorbed into conv2 (two accumulating
            # matmuls: +W2^T a then -W2^T b)
            aT = pool.tile([128, NF], bf16, name="aT", tag="F")
            T2 = pool.tile([128, NF], bf16, name="T2", tag="B")
            for (r0, r1) in HCH:
                sl = slice(128 * r0, 128 * r1)
                nc.vector.tensor_scalar(aT[:, sl], M1[:, sl],
                                        pv[:, 15:16], None, Alu.is_ge)
                nc.vector.tensor_scalar(T2[:, sl], M1[:, sl],
                                        pv[:, 16:17], None, Alu.is_le)

            # ---- conv2 (1x1) ----------------------------------------------
            c2p = pool.tile([128, NF], f32, name="c2p", tag="C")
            s2cols = pool.tile([128, 8], f32, name="s2cols")
            pk2 = pool.tile([128, 2], f32, name="pk2")
            s2sq = pool.tile([128, 8], f32, name="s2sq")
            for bg in range(4):
                pss2 = [pp.tile([128, 1024], f32, name=f"ps2{bg}_{j}", tag=f"ps{j}")
                        for j in range(2)]
                for t in range(2):
                    for j in range(2):
                        for h in range(2):
                            q = 4 * bg + 2 * j + h
                            src = aT if t == 0 else T2
                            nc.tensor.matmul(
                                pss2[j][:, 512 * h:512 * (h + 1)],
                                lhsT=w2s[:, 128 * t:128 * (t + 1)],
                                rhs=src[:, 512 * q:512 * (q + 1)],
                                start=(t == 0), stop=(t == 1),
                                skip_group_check=True)
                for j in range(2):
                    dq = 2 * bg + j
                    nc.scalar.activation(out=c2p[:, 1024 * dq:1024 * (dq + 1)],
                                         in_=pss2[j][:, :], func=Act.Copy,
                                         bias=0.0, scale=1.0,
                                         accum_out=s2cols[:, dq:dq + 1])
                    nc.scalar.activation(out=sqs[:, 1024 * j:1024 * (j + 1)],
                                         in_=pss2[j][:, :], func=Act.Square,
                                         accum_out=s2sq[:, dq:dq + 1])
            nc.vector.tensor_reduce(out=pk2[:, 1:2], in_=s2sq[:, :],
                                    axis=mybir.AxisListType.X, op=Alu.add)
            nc.vector.tensor_reduce(out=pk2[:, 0:1], in_=s2cols[:, :],
                                    axis=mybir.AxisListType.X, op=Alu.add)

            # ---- AllReduce branch-2 stats ---------------------------------
            cc2i = dp.tile([128, 2], f32, name="cc2i")
            cc2o = dp.tile([128, 2], f32, name="cc2o", addr_space="Shared")
            nc.gpsimd.dma_start(out=cc2i[:, :], in_=pk2[:, :])
            nc.gpsimd.collective_compute("AllReduce", Alu.add, replica_groups=RG,
                                         ins=[cc2i[:, :].opt()],
                                         outs=[cc2o[:, :].opt()])
            ar2 = pool.tile([128, 4], f32, name="ar2")
            cc2r = cc2o[:, :].rearrange("(h c) s -> c h s", h=2)
            nc.sync.dma_start(out=ar2[0:64, :].rearrange("c (h s) -> c h s", h=2),
                              in_=cc2r)
            nc.sync.dma_start(out=ar2[64:128, :].rearrange("c (h s) -> c h s", h=2),
                              in_=cc2r)

            sm2 = pool.tile([128, 24], f32, name="sm2")
            # pv col6 = gamma2, col7 = beta2+b22+b13, col8 = p2, col10 = b23
            _post_bn_math(nc, sm2, ar2, pv[:, 6:7], pv[:, 7:8], pv[:, 13:14], pv[:, 14:15])

            # ---- branch-2 apply: u2 = (k2-muk2)*G2 + beta2eff + M1;
            #      out = max(u2+b23, p2*u2+b23) = prelu(u2)+b23 -------------
            w2b = pool.tile([128, NF], f32, name="w2b", tag="D")
            u2 = pool.tile([128, NF], f32, name="u2", tag="E")
            q2 = pool.tile([128, NF], f32, name="q2", tag="C")
            outv = pool.tile([128, NF], f32, name="outv", tag="D")
            o3 = outv.rearrange("p (r c) -> p r c", c=128)
            for (r0, r1) in HCH:
                sl = slice(128 * r0, 128 * r1)
                nc.vector.tensor_scalar(w2b[:, sl], c2p[:, sl],
                                        sm2[:, 20:21], sm2[:, 19:20],
                                        Alu.subtract, Alu.mult)
                nc.vector.tensor_tensor(out=u2[:, sl], in0=w2b[:, sl],
                                        in1=M1[:, sl], op=Alu.add)
                nc.vector.tensor_scalar(q2[:, sl], u2[:, sl],
                                        pv[:, 8:9], pv[:, 17:18],
                                        Alu.mult, Alu.add)
            for (r0, r1) in HCH:
                sl = slice(128 * r0, 128 * r1)
                nc.vector.tensor_tensor(out=outv[:, sl], in0=u2[:, sl],
                                        in1=q2[:, sl], op=Alu.max)

            # ---- store ----------------------------------------------------
            for (r0, r1) in RCH:
                nc.sync.dma_start(out=out_t[:, r0:r1, :], in_=o3[0:64, r0:r1, :])
                nc.sync.dma_start(out=out_t[:, 64 + r0:64 + r1, :],
                                  in_=o3[64:128, r0:r1, :])
    return nc


@functools.lru_cache(maxsize=1)
def get_nc():
    nc = build_nc()
    nc.finalize()   # run_bass_kernel_spmd/bass2jax expects a finalized program
    return nc


def _bf16(a):
    import ml_dtypes
    return a.astype(ml_dtypes.bfloat16)


def host_prep(inputs):
    """Build the small derived device inputs from the full problem inputs."""
    w1 = np.asarray(inputs["w1"], np.float32)     # (64, 16, 3, 3)
    w2 = np.asarray(inputs["w2"], np.float32)     # (64, 64, 1, 1)

    a1 = np.mean(np.abs(w1), axis=(1, 2, 3)).astype(np.float32)      # (64,)
    bw1 = np.asarray(_bf16(np.sign(w1)))          # pure +-1, exact in bf16
    a2 = np.mean(np.abs(w2), axis=(1, 2, 3)).astype(np.float32)
    bw2 = np.asarray(_bf16(np.sign(w2)))

    # conv1 lhsT: [K=128 (in-ch x half), 9 taps, M=128 (out-ch x half)]
    w1t = np.zeros((128, 9, 128), dtype=bw1.dtype)
    for o in range(C):
        g = o // 16
        for ir in range(16):
            i_abs = g * 16 + ir
            for t, (dy, dx) in enumerate(TAPS):
                val = bw1[o, ir, dy, dx]
                w1t[i_abs, t, o] = val
                w1t[64 + i_abs, t, 64 + o] = val
    w1t = w1t.reshape(128, 9 * 128)

    w2t = np.zeros((128, 256), dtype=bw2.dtype)
    w2t[0:64, 0:64] = bw2[:, :, 0, 0].T
    w2t[64:128, 64:128] = bw2[:, :, 0, 0].T
    w2t[:, 128:256] = -w2t[:, 0:128]

    def vec(name):
        return np.asarray(inputs[name], np.float32).reshape(C)

    cols = np.zeros((C, 20), np.float32)
    cols[:, 0] = vec("b11")
    cols[:, 1] = vec("b13") + vec("b21")
    cols[:, 2] = vec("bn1_gamma")
    cols[:, 3] = vec("bn1_beta") + vec("b12")
    cols[:, 4] = vec("p1")
    cols[:, 5] = 1.0 - vec("p1")
    cols[:, 6] = vec("bn2_gamma")
    cols[:, 7] = vec("bn2_beta") + vec("b22") + vec("b13") + vec("b23")
    cols[:, 8] = vec("p2")
    cols[:, 9] = 1.0 - vec("p2")
    cols[:, 10] = vec("b23")
    cols[:, 11] = a1
    cols[:, 12] = a1 * a1
    cols[:, 13] = a2
    cols[:, 14] = a2 * a2
    theta = 1e-6
    cols[:, 15] = theta - cols[:, 1]    # M1 >= theta - sb2  <=>  t >= theta
    cols[:, 16] = -theta - cols[:, 1]   # M1 <= -theta - sb2 <=>  t <= -theta
    cols[:, 17] = (1.0 - vec("p2")) * vec("b23")
    pvec = np.concatenate([cols, cols], axis=0)   # dup across halves -> [128,12]
    return w1t, w2t, pvec


def kernel(**inputs):
    from concourse.bass_utils import run_bass_kernel_spmd

    x = np.ascontiguousarray(np.asarray(inputs["x"], np.float32))   # (8,64,128,128)
    loss = np.asarray(inputs["loss"], np.float32)
    w1t, w2t, pvec = host_prep(inputs)

    nc = get_nc()
    in_maps = [{"x": np.ascontiguousarray(x[i]), "w1t": w1t, "w2t": w2t,
                "pvec": pvec} for i in range(NCORES)]
    res = run_bass_kernel_spmd(nc, in_maps, core_ids=list(range(NCORES)))
    out = np.stack([np.asarray(res.results[i]["out"]) for i in range(NCORES)])
    return out, loss


# revision 17
# speedup vs baseline: 1.1255x; 1.0270x over previous
"""Trainium2 Bass kernel for nn_BasicBlock (ReActNet-style binary basic block).

Strategy: data-parallel over batch (8 images -> 8 NeuronCores). All compute for
one image is local except the BatchNorm batch statistics, which are reduced
across cores with two tiny (128x2 fp32) AllReduces inside a single kernel launch.

Per-core layout ("folded"): the 64-channel image is split into two height-halves
stacked on the 128 SBUF partitions (partition c = channel c rows 0..63,
partition 64+c = channel c rows 64..127). The 3x3 conv input is stored padded as
66 rows x 130 cols per partition (1-pixel halo); both halves share identical
access-pattern offsets, so one matmul stream computes both halves using
block-diagonal (duplicated) weights.

Binary ops are exact in bf16: activations are sign() in {-1,0,+1}; weights are
alpha*sign(w) with per-output-channel alpha. bf16 rounding of alpha is a pure
per-channel scale, which BatchNorm divides right back out, so conv in bf16 with
fp32 PSUM accumulation reproduces the fp32 reference exactly.

Bias/PReLU algebra is folded into per-channel scalars so the whole post-conv
chain is 4 fused passes:
  u = (c * g') + residual          [DVE scalar_tensor_tensor]
  r = Relu(u + Bp)                 [ACT, per-partition bias]
  q = (u * p) + (p*Bp [+ b23])     [DVE tensor_scalar dual-op]
  M = (r * (1-p)) + q              [GPSIMD scalar_tensor_tensor]
which equals prelu(u + Bp) [+ b23] since prelu(z) = (1-p)*relu(z) + p*z.
"""

import functools
import numpy as np

import concourse.bass as bass
import concourse.bacc as bacc
import concourse.tile as tile
from concourse import mybir

EPS = 1e-5
B, C, H = 8, 64, 128
NCORES = 8
PAD = 130          # padded row length (128 + 2)
ROWS = 66          # padded rows per half (64 + 2 halo)
XF = ROWS * PAD    # 8580 padded elements per partition
NF = 64 * 128      # 8192 dense elements per partition (half image)
NTOT = B * H * H   # 131072: count per channel for batch stats
TAPS = [(dy, dx) for dy in range(3) for dx in range(3)]

f32 = mybir.dt.float32
bf16 = mybir.dt.bfloat16
Alu = mybir.AluOpType
Act = mybir.ActivationFunctionType


def _post_bn_math(nc, sm, ar, gamma_col, beta_col, alpha_col, alphasq_col):
    """Tiny per-partition ([128,1]) math turning AllReduced integer count
    sums into per-channel scalars. The conv ran with pure sign(+-1) weights,
    so sums are exact integer counts; alpha is applied here in fp32.

    Writes sm cols: 2 = muk (mean in count units), 19 = G (gamma*rsqrt*alpha).
    The apply pass computes (k - muk)*G == gamma*(c - mu)*rsqrt(var+eps),
    matching the reference's operation order.
    """
    TT = nc.vector.tensor_tensor
    TS = nc.vector.tensor_scalar

    def c(i):
        return sm[:, i:i + 1]

    TT(out=c(0), in0=ar[:, 0:1], in1=ar[:, 2:3], op=Alu.add)      # S1 counts
    TT(out=c(1), in0=ar[:, 1:2], in1=ar[:, 3:4], op=Alu.add)      # S2 counts
    TS(c(2), c(0), 1.0 / NTOT, None, Alu.mult)                    # muk
    TS(c(3), c(1), 1.0 / NTOT, None, Alu.mult)                    # E[k^2]
    TS(c(4), c(2), c(2), None, Alu.mult)                          # muk^2
    TT(out=c(5), in0=c(3), in1=c(4), op=Alu.subtract)             # vark
    TT(out=c(6), in0=c(5), in1=alphasq_col, op=Alu.mult)          # var
    TS(c(6), c(6), EPS, None, Alu.add)                            # var+eps
    nc.scalar.activation(out=c(7), in_=c(6), func=Act.Sqrt)       # ~sqrt
    nc.vector.reciprocal(out=c(8), in_=c(7))                      # y0 ~ rsqrt
    # one Newton step: y = y0*(1.5 - 0.5*(var+eps)*y0^2)
    TS(c(9), c(8), c(8), None, Alu.mult)                          # y0^2
    TT(out=c(10), in0=c(9), in1=c(6), op=Alu.mult)                # v*y0^2
    TS(c(11), c(10), -0.5, 1.5, Alu.mult, Alu.add)                # 1.5-0.5*
    TT(out=c(12), in0=c(8), in1=c(11), op=Alu.mult)               # y (rsqrt)
    TT(out=c(13), in0=c(12), in1=gamma_col, op=Alu.mult)          # g'
    TT(out=c(19), in0=c(13), in1=alpha_col, op=Alu.mult)          # G = g'*alpha
    # fold the additive bias beta into the subtracted mean so the residual
    # add needs no per-channel scalar: (k - (muk - beta/G))*G = (k-muk)*G+beta
    nc.vector.reciprocal(out=c(14), in_=c(19))                    # 1/G
    TT(out=c(15), in0=beta_col, in1=c(14), op=Alu.mult)           # beta/G
    TT(out=c(20), in0=c(2), in1=c(15), op=Alu.subtract)           # muk'


def build_nc():
    nc = bacc.Bacc(None, num_devices=NCORES)

    x_t = nc.declare_dram_parameter("x", [C, H, H], f32, isOutput=False)
    w1_t = nc.declare_dram_parameter("w1t", [128, 9 * 128], bf16, isOutput=False)
    w2_t = nc.declare_dram_parameter("w2t", [128, 256], bf16, isOutput=False)
    pv_t = nc.declare_dram_parameter("pvec", [128, 20], f32, isOutput=False)
    out_t = nc.declare_dram_parameter("out", [C, H, H], f32, isOutput=True)

    RG = [list(range(NCORES))]
    # row-chunk boundaries for chunked full-image passes
    RCH = [(0, 16), (16, 32), (32, 48), (48, 64)]
    HCH = [(0, 32), (32, 64)]

    with tile.TileContext(nc, num_cores=NCORES) as tc:
        with tc.tile_pool(name="main", bufs=1) as pool, \
             tc.tile_pool(name="pp", bufs=2, space="PSUM") as pp, \
             tc.tile_pool(name="dp", bufs=1, space="DRAM") as dp:

            w1s = pool.tile([128, 9 * 128], bf16, name="w1s")
            w2s = pool.tile([128, 256], bf16, name="w2s")
            pv = pool.tile([128, 20], f32, name="pv")
            nc.sync.dma_start(out=w1s[:, :], in_=w1_t[:, :])
            nc.sync.dma_start(out=w2s[:, :], in_=w2_t[:, :])
            nc.sync.dma_start(out=pv[:, :], in_=pv_t[:, :])
            # preload the sqrt ACT table set now, not in the middle of the
            # AllReduce latency window
            warmsq = pool.tile([128, 1], f32, name="warmsq")
            nc.scalar.activation(out=warmsq[:, :], in_=pv[:, 12:13], func=Act.Sqrt)

            # ---- load x into padded/folded layout -------------------------
            xpad = pool.tile([128, XF], f32, name="xpad", tag="A")
            xp3 = xpad.rearrange("p (r c) -> p r c", c=PAD)
            # zero borders: top halo row of half0, bottom halo row of half1,
            # left and right halo columns
            nc.vector.memset(xp3[0:64, 0:1, :], 0.0)
            nc.vector.memset(xp3[64:128, 65:66, :], 0.0)
            nc.vector.memset(xp3[:, :, 0:1], 0.0)
            nc.vector.memset(xp3[:, :, 129:130], 0.0)
            # chunk boundaries aligned with the sign/conv chunks below so the
            # pipeline can start convolving while later rows still stream in
            for (d0, d1) in [(0, 19), (19, 35), (35, 51), (51, 66)]:
                # half0 partitions: padded row r holds image row r-1
                h0, h1 = max(d0, 1), d1
                nc.sync.dma_start(out=xp3[0:64, h0:h1, 1:129],
                                  in_=x_t[:, h0 - 1:h1 - 1, :])
                # half1 partitions: padded row r holds image row 63+r
                g0, g1 = d0, min(d1, 65)
                nc.sync.dma_start(out=xp3[64:128, g0:g1, 1:129],
                                  in_=x_t[:, 63 + g0:63 + g1, :])

            # ---- T1 = sign(x + b11), padded, bf16 -------------------------
            T1 = pool.tile([128, XF], bf16, name="T1", tag="B")
            T13 = T1.rearrange("p (r c) -> p r c", c=PAD)
            wps = pp.tile([128, 512], f32, name="wps", tag="ps0")
            for (r0, r1) in [(0, 19), (19, 35), (35, 51), (51, 66)]:
                nc.scalar.activation(out=T13[:, r0:r1, :], in_=xp3[:, r0:r1, :],
                                     func=Act.Sign, bias=pv[:, 0:1])
                # HAM warmup: keep the PE busy during the load/sign phase so
                # conv1 runs at 2.4 GHz from its first matmul
                for _ in range(8):
                    nc.tensor.matmul(wps[:, :], lhsT=w1s[:, 0:128],
                                     rhs=T13[:, r0:r0 + 4, 0:128],
                                     start=True, stop=True,
                                     skip_group_check=True)
            # restore zero borders (sign(0+b11) may be nonzero there)
            nc.vector.memset(T13[0:64, 0:1, :], 0.0)
            nc.vector.memset(T13[64:128, 65:66, :], 0.0)
            nc.vector.memset(T13[:, :, 0:1], 0.0)
            nc.vector.memset(T13[:, :, 129:130], 0.0)

            # ---- conv1: 9 accumulating matmuls per 512-col chunk ----------
            c1p = pool.tile([128, NF], f32, name="c1p", tag="C")
            s1cols = pool.tile([128, 8], f32, name="s1cols")
            pk1 = pool.tile([128, 2], f32, name="pk1")
            s1sq = pool.tile([128, 8], f32, name="s1sq")
            sqs = pool.tile([128, 2 * 1024], bf16, name="sqs")
            # 8 double-chunks of 1024 cols; each psum tile spans 2 banks and
            # is drained by ONE copy + ONE square (halves ACT op count)
            for bg in range(4):
                pss = [pp.tile([128, 1024], f32, name=f"ps{bg}_{j}", tag=f"ps{j}")
                       for j in range(2)]
                # taps outer: the stationary operand is reused across the
                # group, so LDWEIGHTS amortizes and the PE gets dense
                # back-to-back matmul bursts (keeps HAM un-throttled)
                for t, (dy, dx) in enumerate(TAPS):
                    for j in range(2):
                        for h in range(2):
                            q = 4 * bg + 2 * j + h
                            nc.tensor.matmul(
                                pss[j][:, 512 * h:512 * (h + 1)],
                                lhsT=w1s[:, t * 128:(t + 1) * 128],
                                rhs=T13[:, 4 * q + dy:4 * q + dy + 4, dx:dx + 128],
                                start=(t == 0), stop=(t == 8),
                                skip_group_check=True)
                for j in range(2):
                    dq = 2 * bg + j
                    nc.scalar.activation(out=c1p[:, 1024 * dq:1024 * (dq + 1)],
                                         in_=pss[j][:, :], func=Act.Copy,
                                         bias=0.0, scale=1.0,
                                         accum_out=s1cols[:, dq:dq + 1])
                    nc.scalar.activation(out=sqs[:, 1024 * j:1024 * (j + 1)],
                                         in_=pss[j][:, :], func=Act.Square,
                                         accum_out=s1sq[:, dq:dq + 1])

            nc.vector.tensor_reduce(out=pk1[:, 1:2], in_=s1sq[:, :],
                                    axis=mybir.AxisListType.X, op=Alu.add)
            nc.vector.tensor_reduce(out=pk1[:, 0:1], in_=s1cols[:, :],
                                    axis=mybir.AxisListType.X, op=Alu.add)

            # ---- AllReduce branch-1 stats ---------------------------------
            cc1i = dp.tile([128, 2], f32, name="cc1i")
            cc1o = dp.tile([128, 2], f32, name="cc1o", addr_space="Shared")
            nc.gpsimd.dma_start(out=cc1i[:, :], in_=pk1[:, :])
            nc.gpsimd.collective_compute("AllReduce", Alu.add, replica_groups=RG,
                                         ins=[cc1i[:, :].opt()],
                                         outs=[cc1o[:, :].opt()])
            ar1 = pool.tile([128, 4], f32, name="ar1")
            cc1r = cc1o[:, :].rearrange("(h c) s -> c h s", h=2)
            nc.sync.dma_start(out=ar1[0:64, :].rearrange("c (h s) -> c h s", h=2),
                              in_=cc1r)
            nc.sync.dma_start(out=ar1[64:128, :].rearrange("c (h s) -> c h s", h=2),
                              in_=cc1r)

            sm1 = pool.tile([128, 24], f32, name="sm1")
            # pv col2 = gamma1, col3 = beta1+b12, col4 = p1
            _post_bn_math(nc, sm1, ar1, pv[:, 2:3], pv[:, 3:4], pv[:, 11:12], pv[:, 12:13])

            # ---- branch-1 apply: bn = (k-muk)*G; u = bn+beta12+x;
            #      M1 = prelu(u) = max(u, p1*u) ----------------------------
            w1b = pool.tile([128, NF], f32, name="w1b", tag="D")
            w13 = w1b.rearrange("p (r c) -> p r c", c=128)
            u = pool.tile([128, NF], f32, name="u", tag="E")
            u3 = u.rearrange("p (r c) -> p r c", c=128)
            M1 = pool.tile([128, NF], f32, name="M1", tag="A")
            M13 = M1.rearrange("p (r c) -> p r c", c=128)
            for (r0, r1) in HCH:
                sl = slice(128 * r0, 128 * r1)
                nc.vector.tensor_scalar(w1b[:, sl], c1p[:, sl],
                                        sm1[:, 20:21], sm1[:, 19:20],
                                        Alu.subtract, Alu.mult)
                nc.vector.tensor_tensor(out=u3[:, r0:r1, :], in0=w13[:, r0:r1, :],
                                        in1=xp3[:, 1 + r0:1 + r1, 1:129], op=Alu.add)
            for (r0, r1) in HCH:
                sl = slice(128 * r0, 128 * r1)
                nc.vector.scalar_tensor_tensor(out=M1[:, sl], in0=u[:, sl],
                                               scalar=pv[:, 4:5], in1=u[:, sl],
                                               op0=Alu.mult, op1=Alu.max)

            # ---- T2 = -deadzone_sign(M1 + b13 + b21), dense bf16 ----------
            # deadzone: |t| < theta -> 0 (theta ~ fp32 noise floor), so the
            # reference's exact-zero sign inputs stay zero. Negation is folded
            # into the sign of alpha2 on the host.
            # a = (M1 >= theta - sb2); b = (M1 <= -theta - sb2); the
            # subtraction T2 = a - b is absorbed into conv2 (two accumulating
            # matmuls: +W2^T a then -W2^T b)
            aT = pool.tile([128, NF], bf16, name="aT", tag="F")
            T2 = pool.tile([128, NF], bf16, name="T2", tag="B")
            for (r0, r1) in HCH:
                sl = slice(128 * r0, 128 * r1)
                nc.vector.tensor_scalar(aT[:, sl], M1[:, sl],
                                        pv[:, 15:16], None, Alu.is_ge)
                nc.vector.tensor_scalar(T2[:, sl], M1[:, sl],
                                        pv[:, 16:17], None, Alu.is_le)

            # ---- conv2 (1x1) ----------------------------------------------
            c2p = pool.tile([128, NF], f32, name="c2p", tag="C")
            s2cols = pool.tile([128, 8], f32, name="s2cols")
            pk2 = pool.tile([128, 2], f32, name="pk2")
            s2sq = pool.tile([128, 8], f32, name="s2sq")
            for bg in range(4):
                pss2 = [pp.tile([128, 1024], f32, name=f"ps2{bg}_{j}", tag=f"ps{j}")
                        for j in range(2)]
                for t in range(2):
                    for j in range(2):
                        for h in range(2):
                            q = 4 * bg + 2 * j + h
                            src = aT if t == 0 else T2
                            nc.tensor.matmul(
                                pss2[j][:, 512 * h:512 * (h + 1)],
                                lhsT=w2s[:, 128 * t:128 * (t + 1)],
                                rhs=src[:, 512 * q:512 * (q + 1)],
                                start=(t == 0), stop=(t == 1),
                                skip_group_check=True)
                for j in range(2):
                    dq = 2 * bg + j
                    nc.scalar.activation(out=c2p[:, 1024 * dq:1024 * (dq + 1)],
                                         in_=pss2[j][:, :], func=Act.Copy,
                                         bias=0.0, scale=1.0,
                                         accum_out=s2cols[:, dq:dq + 1])
                    nc.scalar.activation(out=sqs[:, 1024 * j:1024 * (j + 1)],
                                         in_=pss2[j][:, :], func=Act.Square,
                                         accum_out=s2sq[:, dq:dq + 1])
            nc.vector.tensor_reduce(out=pk2[:, 1:2], in_=s2sq[:, :],
                                    axis=mybir.AxisListType.X, op=Alu.add)
            nc.vector.tensor_reduce(out=pk2[:, 0:1], in_=s2cols[:, :],
                                    axis=mybir.AxisListType.X, op=Alu.add)

            # ---- AllReduce branch-2 stats ---------------------------------
            cc2i = dp.tile([128, 2], f32, name="cc2i")
            cc2o = dp.tile([128, 2], f32, name="cc2o", addr_space="Shared")
            nc.gpsimd.dma_start(out=cc2i[:, :], in_=pk2[:, :])
            nc.gpsimd.collective_compute("AllReduce", Alu.add, replica_groups=RG,
                                         ins=[cc2i[:, :].opt()],
                                         outs=[cc2o[:, :].opt()])
            ar2 = pool.tile([128, 4], f32, name="ar2")
            cc2r = cc2o[:, :].rearrange("(h c) s -> c h s", h=2)
            nc.sync.dma_start(out=ar2[0:64, :].rearrange("c (h s) -> c h s", h=2),
                              in_=cc2r)
            nc.sync.dma_start(out=ar2[64:128, :].rearrange("c (h s) -> c h s", h=2),
                              in_=cc2r)

            sm2 = pool.tile([128, 24], f32, name="sm2")
            # pv col6 = gamma2, col7 = beta2+b22+b13, col8 = p2, col10 = b23
            _post_bn_math(nc, sm2, ar2, pv[:, 6:7], pv[:, 7:8], pv[:, 13:14], pv[:, 14:15])

            # ---- branch-2 apply: u2 = (k2-muk2)*G2 + beta2eff + M1;
            #      out = max(u2+b23, p2*u2+b23) = prelu(u2)+b23 -------------
            w2b = pool.tile([128, NF], f32, name="w2b", tag="D")
            u2 = pool.tile([128, NF], f32, name="u2", tag="E")
            q2 = pool.tile([128, NF], f32, name="q2", tag="C")
            outv_a = pool.tile([128, NF // 2], f32, name="outv_a", tag="D")
            outv_b = pool.tile([128, NF // 2], f32, name="outv_b", tag="F")
            for (r0, r1) in HCH:
                sl = slice(128 * r0, 128 * r1)
                nc.vector.tensor_scalar(w2b[:, sl], c2p[:, sl],
                                        sm2[:, 20:21], sm2[:, 19:20],
                                        Alu.subtract, Alu.mult)
                nc.vector.tensor_tensor(out=u2[:, sl], in0=w2b[:, sl],
                                        in1=M1[:, sl], op=Alu.add)
                nc.vector.tensor_scalar(q2[:, sl], u2[:, sl],
                                        pv[:, 8:9], pv[:, 17:18],
                                        Alu.mult, Alu.add)
            for i, (r0, r1) in enumerate(HCH):
                sl = slice(128 * r0, 128 * r1)
                ov = outv_a if i == 0 else outv_b
                nc.vector.tensor_tensor(out=ov[:, :], in0=u2[:, sl],
                                        in1=q2[:, sl], op=Alu.max)

            # ---- store (per half-tile so DMA overlaps the second max) -----
            for i, ov in enumerate((outv_a, outv_b)):
                ov3 = ov.rearrange("p (r c) -> p r c", c=128)
                for (r0, r1) in [(0, 16), (16, 32)]:
                    nc.sync.dma_start(out=out_t[:, 32 * i + r0:32 * i + r1, :],
                                      in_=ov3[0:64, r0:r1, :])
                    nc.sync.dma_start(out=out_t[:, 64 + 32 * i + r0:64 + 32 * i + r1, :],
                                      in_=ov3[64:128, r0:r1, :])
    return nc


@functools.lru_cache(maxsize=1)
def get_nc():
    nc = build_nc()
    nc.finalize()   # run_bass_kernel_spmd/bass2jax expects a finalized program
    return nc


def _bf16(a):
    import ml_dtypes
    return a.astype(ml_dtypes.bfloat16)


def host_prep(inputs):
    """Build the small derived device inputs from the full problem inputs."""
    w1 = np.asarray(inputs["w1"], np.float32)     # (64, 16, 3, 3)
    w2 = np.asarray(inputs["w2"], np.float32)     # (64, 64, 1, 1)

    a1 = np.mean(np.abs(w1), axis=(1, 2, 3)).astype(np.float32)      # (64,)
    bw1 = np.asarray(_bf16(np.sign(w1)))          # pure +-1, exact in bf16
    a2 = np.mean(np.abs(w2), axis=(1, 2, 3)).astype(np.float32)
    bw2 = np.asarray(_bf16(np.sign(w2)))

    # conv1 lhsT: [K=128 (in-ch x half), 9 taps, M=128 (out-ch x half)]
    w1t = np.zeros((128, 9, 128), dtype=bw1.dtype)
    for o in range(C):
        g = o // 16
        for ir in range(16):
            i_abs = g * 16 + ir
            for t, (dy, dx) in enumerate(TAPS):
                val = bw1[o, ir, dy, dx]
                w1t[i_abs, t, o] = val
                w1t[64 + i_abs, t, 64 + o] = val
    w1t = w1t.reshape(128, 9 * 128)

    w2t = np.zeros((128, 256), dtype=bw2.dtype)
    w2t[0:64, 0:64] = bw2[:, :, 0, 0].T
    w2t[64:128, 64:128] = bw2[:, :, 0, 0].T
    w2t[:, 128:256] = -w2t[:, 0:128]

    def vec(name):
        return np.asarray(inputs[name], np.float32).reshape(C)

    cols = np.zeros((C, 20), np.float32)
    cols[:, 0] = vec("b11")
    cols[:, 1] = vec("b13") + vec("b21")
    cols[:, 2] = vec("bn1_gamma")
    cols[:, 3] = vec("bn1_beta") + vec("b12")
    cols[:, 4] = vec("p1")
    cols[:, 5] = 1.0 - vec("p1")
    cols[:, 6] = vec("bn2_gamma")
    cols[:, 7] = vec("bn2_beta") + vec("b22") + vec("b13") + vec("b23")
    cols[:, 8] = vec("p2")
    cols[:, 9] = 1.0 - vec("p2")
    cols[:, 10] = vec("b23")
    cols[:, 11] = a1
    cols[:, 12] = a1 * a1
    cols[:, 13] = a2
    cols[:, 14] = a2 * a2
    theta = 1e-6
    cols[:, 15] = theta - cols[:, 1]    # M1 >= theta - sb2  <=>  t >= theta
    cols[:, 16] = -theta - cols[:, 1]   # M1 <= -theta - sb2 <=>  t <= -theta
    cols[:, 17] = (1.0 - vec("p2")) * vec("b23")
    pvec = np.concatenate([cols, cols], axis=0)   # dup across halves -> [128,12]
    return w1t, w2t, pvec


def kernel(**inputs):
    from concourse.bass_utils import run_bass_kernel_spmd

    x = np.ascontiguousarray(np.asarray(inputs["x"], np.float32))   # (8,64,128,128)
    loss = np.asarray(inputs["loss"], np.float32)
    w1t, w2t, pvec = host_prep(inputs)

    nc = get_nc()
    in_maps = [{"x": np.ascontiguousarray(x[i]), "w1t": w1t, "w2t": w2t,
                "pvec": pvec} for i in range(NCORES)]
    res = run_bass_kernel_spmd(nc, in_maps, core_ids=list(range(NCORES)))
    out = np.stack([np.asarray(res.results[i]["out"]) for i in range(NCORES)])
    return out, loss


# revision 18
# speedup vs baseline: 1.1257x; 1.0001x over previous
"""Trainium2 Bass kernel for nn_BasicBlock (ReActNet-style binary basic block).

Strategy: data-parallel over batch (8 images -> 8 NeuronCores). All compute for
one image is local except the BatchNorm batch statistics, which are reduced
across cores with two tiny (128x2 fp32) AllReduces inside a single kernel launch.

Per-core layout ("folded"): the 64-channel image is split into two height-halves
stacked on the 128 SBUF partitions (partition c = channel c rows 0..63,
partition 64+c = channel c rows 64..127). The 3x3 conv input is stored padded as
66 rows x 130 cols per partition (1-pixel halo); both halves share identical
access-pattern offsets, so one matmul stream computes both halves using
block-diagonal (duplicated) weights.

Binary ops are exact in bf16: activations are sign() in {-1,0,+1}; weights are
alpha*sign(w) with per-output-channel alpha. bf16 rounding of alpha is a pure
per-channel scale, which BatchNorm divides right back out, so conv in bf16 with
fp32 PSUM accumulation reproduces the fp32 reference exactly.

Bias/PReLU algebra is folded into per-channel scalars so the whole post-conv
chain is 4 fused passes:
  u = (c * g') + residual          [DVE scalar_tensor_tensor]
  r = Relu(u + Bp)                 [ACT, per-partition bias]
  q = (u * p) + (p*Bp [+ b23])     [DVE tensor_scalar dual-op]
  M = (r * (1-p)) + q              [GPSIMD scalar_tensor_tensor]
which equals prelu(u + Bp) [+ b23] since prelu(z) = (1-p)*relu(z) + p*z.
"""

import functools
import numpy as np

import concourse.bass as bass
import concourse.bacc as bacc
import concourse.tile as tile
from concourse import mybir

EPS = 1e-5
B, C, H = 8, 64, 128
NCORES = 8
PAD = 130          # padded row length (128 + 2)
ROWS = 66          # padded rows per half (64 + 2 halo)
XF = ROWS * PAD    # 8580 padded elements per partition
NF = 64 * 128      # 8192 dense elements per partition (half image)
NTOT = B * H * H   # 131072: count per channel for batch stats
TAPS = [(dy, dx) for dy in range(3) for dx in range(3)]

f32 = mybir.dt.float32
bf16 = mybir.dt.bfloat16
Alu = mybir.AluOpType
Act = mybir.ActivationFunctionType


def _post_bn_math(nc, sm, ar, gamma_col, beta_col, alpha_col, alphasq_col):
    """Tiny per-partition ([128,1]) math turning AllReduced integer count
    sums into per-channel scalars. The conv ran with pure sign(+-1) weights,
    so sums are exact integer counts; alpha is applied here in fp32.

    Writes sm cols: 2 = muk (mean in count units), 19 = G (gamma*rsqrt*alpha).
    The apply pass computes (k - muk)*G == gamma*(c - mu)*rsqrt(var+eps),
    matching the reference's operation order.
    """
    TT = nc.vector.tensor_tensor
    TS = nc.vector.tensor_scalar

    def c(i):
        return sm[:, i:i + 1]

    TT(out=c(0), in0=ar[:, 0:1], in1=ar[:, 2:3], op=Alu.add)      # S1 counts
    TT(out=c(1), in0=ar[:, 1:2], in1=ar[:, 3:4], op=Alu.add)      # S2 counts
    TS(c(2), c(0), 1.0 / NTOT, None, Alu.mult)                    # muk
    TS(c(3), c(1), 1.0 / NTOT, None, Alu.mult)                    # E[k^2]
    TS(c(4), c(2), c(2), None, Alu.mult)                          # muk^2
    TT(out=c(5), in0=c(3), in1=c(4), op=Alu.subtract)             # vark
    TT(out=c(6), in0=c(5), in1=alphasq_col, op=Alu.mult)          # var
    TS(c(6), c(6), EPS, None, Alu.add)                            # var+eps
    nc.scalar.activation(out=c(7), in_=c(6), func=Act.Sqrt)       # ~sqrt
    nc.vector.reciprocal(out=c(8), in_=c(7))                      # y0 ~ rsqrt
    # one Newton step: y = y0*(1.5 - 0.5*(var+eps)*y0^2)
    TS(c(9), c(8), c(8), None, Alu.mult)                          # y0^2
    TT(out=c(10), in0=c(9), in1=c(6), op=Alu.mult)                # v*y0^2
    TS(c(11), c(10), -0.5, 1.5, Alu.mult, Alu.add)                # 1.5-0.5*
    TT(out=c(12), in0=c(8), in1=c(11), op=Alu.mult)               # y (rsqrt)
    TT(out=c(13), in0=c(12), in1=gamma_col, op=Alu.mult)          # g'
    TT(out=c(19), in0=c(13), in1=alpha_col, op=Alu.mult)          # G = g'*alpha
    # fold the additive bias beta into the subtracted mean so the residual
    # add needs no per-channel scalar: (k - (muk - beta/G))*G = (k-muk)*G+beta
    nc.vector.reciprocal(out=c(14), in_=c(19))                    # 1/G
    TT(out=c(15), in0=beta_col, in1=c(14), op=Alu.mult)           # beta/G
    TT(out=c(20), in0=c(2), in1=c(15), op=Alu.subtract)           # muk'


def build_nc():
    nc = bacc.Bacc(None, num_devices=NCORES)

    x_t = nc.declare_dram_parameter("x", [C, H, H], f32, isOutput=False)
    w1_t = nc.declare_dram_parameter("w1t", [128, 9 * 128], bf16, isOutput=False)
    w2_t = nc.declare_dram_parameter("w2t", [128, 256], bf16, isOutput=False)
    pv_t = nc.declare_dram_parameter("pvec", [128, 20], f32, isOutput=False)
    out_t = nc.declare_dram_parameter("out", [C, H, H], f32, isOutput=True)

    RG = [list(range(NCORES))]
    # row-chunk boundaries for chunked full-image passes
    RCH = [(0, 16), (16, 32), (32, 48), (48, 64)]
    HCH = [(0, 32), (32, 64)]

    with tile.TileContext(nc, num_cores=NCORES) as tc:
        with tc.tile_pool(name="main", bufs=1) as pool, \
             tc.tile_pool(name="pp", bufs=2, space="PSUM") as pp, \
             tc.tile_pool(name="dp", bufs=1, space="DRAM") as dp:

            w1s = pool.tile([128, 9 * 128], bf16, name="w1s")
            w2s = pool.tile([128, 256], bf16, name="w2s")
            pv = pool.tile([128, 20], f32, name="pv")
            nc.sync.dma_start(out=w1s[:, :], in_=w1_t[:, :])
            nc.sync.dma_start(out=w2s[:, :], in_=w2_t[:, :])
            nc.sync.dma_start(out=pv[:, :], in_=pv_t[:, :])
            # preload the sqrt ACT table set now, not in the middle of the
            # AllReduce latency window
            warmsq = pool.tile([128, 1], f32, name="warmsq")
            nc.scalar.activation(out=warmsq[:, :], in_=pv[:, 12:13], func=Act.Sqrt)

            # ---- load x into padded/folded layout -------------------------
            xpad = pool.tile([128, XF], f32, name="xpad", tag="A")
            xp3 = xpad.rearrange("p (r c) -> p r c", c=PAD)
            # zero borders: top halo row of half0, bottom halo row of half1,
            # left and right halo columns
            nc.vector.memset(xp3[0:64, 0:1, :], 0.0)
            nc.vector.memset(xp3[64:128, 65:66, :], 0.0)
            nc.vector.memset(xp3[:, :, 0:1], 0.0)
            nc.vector.memset(xp3[:, :, 129:130], 0.0)
            # chunk boundaries aligned with the sign/conv chunks below so the
            # pipeline can start convolving while later rows still stream in
            for (d0, d1) in [(0, 19), (19, 35), (35, 51), (51, 66)]:
                # half0 partitions: padded row r holds image row r-1
                h0, h1 = max(d0, 1), d1
                nc.sync.dma_start(out=xp3[0:64, h0:h1, 1:129],
                                  in_=x_t[:, h0 - 1:h1 - 1, :])
                # half1 partitions: padded row r holds image row 63+r
                g0, g1 = d0, min(d1, 65)
                nc.sync.dma_start(out=xp3[64:128, g0:g1, 1:129],
                                  in_=x_t[:, 63 + g0:63 + g1, :])

            # ---- T1 = sign(x + b11), padded, bf16 -------------------------
            # T1 split into two overlapping row-tiles (global rows 0..35 and
            # 32..66) so conv1's first bank groups only depend on the first
            # sign pass instead of the whole activation tensor
            T1a = pool.tile([128, 35 * PAD], bf16, name="T1a", tag="B")
            T1b = pool.tile([128, 34 * PAD], bf16, name="T1b", tag="B2")
            Ta3 = T1a.rearrange("p (r c) -> p r c", c=PAD)
            Tb3 = T1b.rearrange("p (r c) -> p r c", c=PAD)
            wps = pp.tile([128, 512], f32, name="wps", tag="ps0")
            for half, (t3, g0, g1) in enumerate([(Ta3, 0, 35), (Tb3, 32, 66)]):
                nc.scalar.activation(out=t3[:, :, :], in_=xp3[:, g0:g1, :],
                                     func=Act.Sign, bias=pv[:, 0:1])
                # zero borders (sign(0+b11) may be nonzero there)
                nc.vector.memset(t3[:, :, 0:1], 0.0)
                nc.vector.memset(t3[:, :, 129:130], 0.0)
                if half == 0:
                    nc.vector.memset(t3[0:64, 0:1, :], 0.0)
                else:
                    nc.vector.memset(t3[64:128, 33:34, :], 0.0)
                # HAM warmup: keep the PE busy during the load/sign phase so
                # conv1 runs at 2.4 GHz from its first matmul
                for _ in range(16):
                    nc.tensor.matmul(wps[:, :], lhsT=w1s[:, 0:128],
                                     rhs=t3[:, 0:4, 0:128],
                                     start=True, stop=True,
                                     skip_group_check=True)

            # ---- conv1: 9 accumulating matmuls per 512-col chunk ----------
            c1p = pool.tile([128, NF], f32, name="c1p", tag="C")
            s1cols = pool.tile([128, 8], f32, name="s1cols")
            pk1 = pool.tile([128, 2], f32, name="pk1")
            s1sq = pool.tile([128, 8], f32, name="s1sq")
            sqs = pool.tile([128, 2 * 1024], bf16, name="sqs")
            # 8 double-chunks of 1024 cols; each psum tile spans 2 banks and
            # is drained by ONE copy + ONE square (halves ACT op count)
            for bg in range(4):
                pss = [pp.tile([128, 1024], f32, name=f"ps{bg}_{j}", tag=f"ps{j}")
                       for j in range(2)]
                # taps outer: the stationary operand is reused across the
                # group, so LDWEIGHTS amortizes and the PE gets dense
                # back-to-back matmul bursts (keeps HAM un-throttled)
                for t, (dy, dx) in enumerate(TAPS):
                    for j in range(2):
                        for h in range(2):
                            q = 4 * bg + 2 * j + h
                            src3 = Ta3 if bg < 2 else Tb3
                            r = 4 * q + dy - (0 if bg < 2 else 32)
                            nc.tensor.matmul(
                                pss[j][:, 512 * h:512 * (h + 1)],
                                lhsT=w1s[:, t * 128:(t + 1) * 128],
                                rhs=src3[:, r:r + 4, dx:dx + 128],
                                start=(t == 0), stop=(t == 8),
                                skip_group_check=True)
                for j in range(2):
                    dq = 2 * bg + j
                    nc.scalar.activation(out=c1p[:, 1024 * dq:1024 * (dq + 1)],
                                         in_=pss[j][:, :], func=Act.Copy,
                                         bias=0.0, scale=1.0,
                                         accum_out=s1cols[:, dq:dq + 1])
                    nc.scalar.activation(out=sqs[:, 1024 * j:1024 * (j + 1)],
                                         in_=pss[j][:, :], func=Act.Square,
                                         accum_out=s1sq[:, dq:dq + 1])

            nc.vector.tensor_reduce(out=pk1[:, 1:2], in_=s1sq[:, :],
                                    axis=mybir.AxisListType.X, op=Alu.add)
            nc.vector.tensor_reduce(out=pk1[:, 0:1], in_=s1cols[:, :],
                                    axis=mybir.AxisListType.X, op=Alu.add)

            # ---- AllReduce branch-1 stats ---------------------------------
            cc1i = dp.tile([128, 2], f32, name="cc1i")
            cc1o = dp.tile([128, 2], f32, name="cc1o", addr_space="Shared")
            nc.gpsimd.dma_start(out=cc1i[:, :], in_=pk1[:, :])
            nc.gpsimd.collective_compute("AllReduce", Alu.add, replica_groups=RG,
                                         ins=[cc1i[:, :].opt()],
                                         outs=[cc1o[:, :].opt()])
            ar1 = pool.tile([128, 4], f32, name="ar1")
            cc1r = cc1o[:, :].rearrange("(h c) s -> c h s", h=2)
            nc.sync.dma_start(out=ar1[0:64, :].rearrange("c (h s) -> c h s", h=2),
                              in_=cc1r)
            nc.sync.dma_start(out=ar1[64:128, :].rearrange("c (h s) -> c h s", h=2),
                              in_=cc1r)

            sm1 = pool.tile([128, 24], f32, name="sm1")
            # pv col2 = gamma1, col3 = beta1+b12, col4 = p1
            _post_bn_math(nc, sm1, ar1, pv[:, 2:3], pv[:, 3:4], pv[:, 11:12], pv[:, 12:13])

            # ---- branch-1 apply: bn = (k-muk)*G; u = bn+beta12+x;
            #      M1 = prelu(u) = max(u, p1*u) ----------------------------
            w1b = pool.tile([128, NF], f32, name="w1b", tag="D")
            w13 = w1b.rearrange("p (r c) -> p r c", c=128)
            u = pool.tile([128, NF], f32, name="u", tag="E")
            u3 = u.rearrange("p (r c) -> p r c", c=128)
            M1 = pool.tile([128, NF], f32, name="M1", tag="A")
            M13 = M1.rearrange("p (r c) -> p r c", c=128)
            for (r0, r1) in HCH:
                sl = slice(128 * r0, 128 * r1)
                nc.vector.tensor_scalar(w1b[:, sl], c1p[:, sl],
                                        sm1[:, 20:21], sm1[:, 19:20],
                                        Alu.subtract, Alu.mult)
                nc.vector.tensor_tensor(out=u3[:, r0:r1, :], in0=w13[:, r0:r1, :],
                                        in1=xp3[:, 1 + r0:1 + r1, 1:129], op=Alu.add)
            for (r0, r1) in HCH:
                sl = slice(128 * r0, 128 * r1)
                nc.vector.scalar_tensor_tensor(out=M1[:, sl], in0=u[:, sl],
                                               scalar=pv[:, 4:5], in1=u[:, sl],
                                               op0=Alu.mult, op1=Alu.max)

            # ---- T2 = -deadzone_sign(M1 + b13 + b21), dense bf16 ----------
            # deadzone: |t| < theta -> 0 (theta ~ fp32 noise floor), so the
            # reference's exact-zero sign inputs stay zero. Negation is folded
            # into the sign of alpha2 on the host.
            # a = (M1 >= theta - sb2); b = (M1 <= -theta - sb2); the
            # subtraction T2 = a - b is absorbed into conv2 (two accumulating
            # matmuls: +W2^T a then -W2^T b)
            aT = pool.tile([128, NF], bf16, name="aT", tag="F")
            T2 = pool.tile([128, NF], bf16, name="T2", tag="B")
            for (r0, r1) in HCH:
                sl = slice(128 * r0, 128 * r1)
                nc.vector.tensor_scalar(aT[:, sl], M1[:, sl],
                                        pv[:, 15:16], None, Alu.is_ge)
                nc.vector.tensor_scalar(T2[:, sl], M1[:, sl],
                                        pv[:, 16:17], None, Alu.is_le)

            # ---- conv2 (1x1) ----------------------------------------------
            c2p = pool.tile([128, NF], f32, name="c2p", tag="C")
            s2cols = pool.tile([128, 8], f32, name="s2cols")
            pk2 = pool.tile([128, 2], f32, name="pk2")
            s2sq = pool.tile([128, 8], f32, name="s2sq")
            for bg in range(4):
                pss2 = [pp.tile([128, 1024], f32, name=f"ps2{bg}_{j}", tag=f"ps{j}")
                        for j in range(2)]
                for t in range(2):
                    for j in range(2):
                        for h in range(2):
                            q = 4 * bg + 2 * j + h
                            src = aT if t == 0 else T2
                            nc.tensor.matmul(
                                pss2[j][:, 512 * h:512 * (h + 1)],
                                lhsT=w2s[:, 128 * t:128 * (t + 1)],
                                rhs=src[:, 512 * q:512 * (q + 1)],
                                start=(t == 0), stop=(t == 1),
                                skip_group_check=True)
                for j in range(2):
                    dq = 2 * bg + j
                    nc.scalar.activation(out=c2p[:, 1024 * dq:1024 * (dq + 1)],
                                         in_=pss2[j][:, :], func=Act.Copy,
                                         bias=0.0, scale=1.0,
                                         accum_out=s2cols[:, dq:dq + 1])
                    nc.scalar.activation(out=sqs[:, 1024 * j:1024 * (j + 1)],
                                         in_=pss2[j][:, :], func=Act.Square,
                                         accum_out=s2sq[:, dq:dq + 1])
            nc.vector.tensor_reduce(out=pk2[:, 1:2], in_=s2sq[:, :],
                                    axis=mybir.AxisListType.X, op=Alu.add)
            nc.vector.tensor_reduce(out=pk2[:, 0:1], in_=s2cols[:, :],
                                    axis=mybir.AxisListType.X, op=Alu.add)

            # ---- AllReduce branch-2 stats ---------------------------------
            cc2i = dp.tile([128, 2], f32, name="cc2i")
            cc2o = dp.tile([128, 2], f32, name="cc2o", addr_space="Shared")
            nc.gpsimd.dma_start(out=cc2i[:, :], in_=pk2[:, :])
            nc.gpsimd.collective_compute("AllReduce", Alu.add, replica_groups=RG,
                                         ins=[cc2i[:, :].opt()],
                                         outs=[cc2o[:, :].opt()])
            ar2 = pool.tile([128, 4], f32, name="ar2")
            cc2r = cc2o[:, :].rearrange("(h c) s -> c h s", h=2)
            nc.sync.dma_start(out=ar2[0:64, :].rearrange("c (h s) -> c h s", h=2),
                              in_=cc2r)
            nc.sync.dma_start(out=ar2[64:128, :].rearrange("c (h s) -> c h s", h=2),
                              in_=cc2r)

            sm2 = pool.tile([128, 24], f32, name="sm2")
            # pv col6 = gamma2, col7 = beta2+b22+b13, col8 = p2, col10 = b23
            _post_bn_math(nc, sm2, ar2, pv[:, 6:7], pv[:, 7:8], pv[:, 13:14], pv[:, 14:15])

            # ---- branch-2 apply: u2 = (k2-muk2)*G2 + beta2eff + M1;
            #      out = max(u2+b23, p2*u2+b23) = prelu(u2)+b23 -------------
            w2b = pool.tile([128, NF], f32, name="w2b", tag="D")
            u2 = pool.tile([128, NF], f32, name="u2", tag="E")
            q2 = pool.tile([128, NF], f32, name="q2", tag="C")
            outv_a = pool.tile([128, NF // 2], f32, name="outv_a", tag="D")
            outv_b = pool.tile([128, NF // 2], f32, name="outv_b", tag="F")
            for (r0, r1) in HCH:
                sl = slice(128 * r0, 128 * r1)
                nc.vector.tensor_scalar(w2b[:, sl], c2p[:, sl],
                                        sm2[:, 20:21], sm2[:, 19:20],
                                        Alu.subtract, Alu.mult)
                nc.vector.tensor_tensor(out=u2[:, sl], in0=w2b[:, sl],
                                        in1=M1[:, sl], op=Alu.add)
                nc.vector.tensor_scalar(q2[:, sl], u2[:, sl],
                                        pv[:, 8:9], pv[:, 17:18],
                                        Alu.mult, Alu.add)
            for i, (r0, r1) in enumerate(HCH):
                sl = slice(128 * r0, 128 * r1)
                ov = outv_a if i == 0 else outv_b
                nc.vector.tensor_tensor(out=ov[:, :], in0=u2[:, sl],
                                        in1=q2[:, sl], op=Alu.max)

            # ---- store (per half-tile so DMA overlaps the second max) -----
            for i, ov in enumerate((outv_a, outv_b)):
                ov3 = ov.rearrange("p (r c) -> p r c", c=128)
                for (r0, r1) in [(0, 16), (16, 32)]:
                    nc.sync.dma_start(out=out_t[:, 32 * i + r0:32 * i + r1, :],
                                      in_=ov3[0:64, r0:r1, :])
                    nc.sync.dma_start(out=out_t[:, 64 + 32 * i + r0:64 + 32 * i + r1, :],
                                      in_=ov3[64:128, r0:r1, :])
    return nc


@functools.lru_cache(maxsize=1)
def get_nc():
    nc = build_nc()
    nc.finalize()   # run_bass_kernel_spmd/bass2jax expects a finalized program
    return nc


def _bf16(a):
    import ml_dtypes
    return a.astype(ml_dtypes.bfloat16)


def host_prep(inputs):
    """Build the small derived device inputs from the full problem inputs."""
    w1 = np.asarray(inputs["w1"], np.float32)     # (64, 16, 3, 3)
    w2 = np.asarray(inputs["w2"], np.float32)     # (64, 64, 1, 1)

    a1 = np.mean(np.abs(w1), axis=(1, 2, 3)).astype(np.float32)      # (64,)
    bw1 = np.asarray(_bf16(np.sign(w1)))          # pure +-1, exact in bf16
    a2 = np.mean(np.abs(w2), axis=(1, 2, 3)).astype(np.float32)
    bw2 = np.asarray(_bf16(np.sign(w2)))

    # conv1 lhsT: [K=128 (in-ch x half), 9 taps, M=128 (out-ch x half)]
    w1t = np.zeros((128, 9, 128), dtype=bw1.dtype)
    for o in range(C):
        g = o // 16
        for ir in range(16):
            i_abs = g * 16 + ir
            for t, (dy, dx) in enumerate(TAPS):
                val = bw1[o, ir, dy, dx]
                w1t[i_abs, t, o] = val
                w1t[64 + i_abs, t, 64 + o] = val
    w1t = w1t.reshape(128, 9 * 128)

    w2t = np.zeros((128, 256), dtype=bw2.dtype)
    w2t[0:64, 0:64] = bw2[:, :, 0, 0].T
    w2t[64:128, 64:128] = bw2[:, :, 0, 0].T
    w2t[:, 128:256] = -w2t[:, 0:128]

    def vec(name):
        return np.asarray(inputs[name], np.float32).reshape(C)

    cols = np.zeros((C, 20), np.float32)
    cols[:, 0] = vec("b11")
    cols[:, 1] = vec("b13") + vec("b21")
    cols[:, 2] = vec("bn1_gamma")
    cols[:, 3] = vec("bn1_beta") + vec("b12")
    cols[:, 4] = vec("p1")
    cols[:, 5] = 1.0 - vec("p1")
    cols[:, 6] = vec("bn2_gamma")
    cols[:, 7] = vec("bn2_beta") + vec("b22") + vec("b13") + vec("b23")
    cols[:, 8] = vec("p2")
    cols[:, 9] = 1.0 - vec("p2")
    cols[:, 10] = vec("b23")
    cols[:, 11] = a1
    cols[:, 12] = a1 * a1
    cols[:, 13] = a2
    cols[:, 14] = a2 * a2
    theta = 1e-6
    cols[:, 15] = theta - cols[:, 1]    # M1 >= theta - sb2  <=>  t >= theta
    cols[:, 16] = -theta - cols[:, 1]   # M1 <= -theta - sb2 <=>  t <= -theta
    cols[:, 17] = (1.0 - vec("p2")) * vec("b23")
    pvec = np.concatenate([cols, cols], axis=0)   # dup across halves -> [128,12]
    return w1t, w2t, pvec


def kernel(**inputs):
    from concourse.bass_utils import run_bass_kernel_spmd

    x = np.ascontiguousarray(np.asarray(inputs["x"], np.float32))   # (8,64,128,128)
    loss = np.asarray(inputs["loss"], np.float32)
    w1t, w2t, pvec = host_prep(inputs)

    nc = get_nc()
    in_maps = [{"x": np.ascontiguousarray(x[i]), "w1t": w1t, "w2t": w2t,
                "pvec": pvec} for i in range(NCORES)]
    res = run_bass_kernel_spmd(nc, in_maps, core_ids=list(range(NCORES)))
    out = np.stack([np.asarray(res.results[i]["out"]) for i in range(NCORES)])
    return out, loss


# revision 20
# speedup vs baseline: 1.2109x; 1.0757x over previous
"""Trainium2 Bass kernel for nn_BasicBlock (ReActNet-style binary basic block).

Strategy: data-parallel over batch (8 images -> 8 NeuronCores). All compute for
one image is local except the BatchNorm batch statistics, which are reduced
across cores with two tiny (128x2 fp32) AllReduces inside a single kernel launch.

Per-core layout ("folded"): the 64-channel image is split into two height-halves
stacked on the 128 SBUF partitions (partition c = channel c rows 0..63,
partition 64+c = channel c rows 64..127). The 3x3 conv input is stored padded as
66 rows x 130 cols per partition (1-pixel halo); both halves share identical
access-pattern offsets, so one matmul stream computes both halves using
block-diagonal (duplicated) weights.

Binary ops are exact in bf16: activations are sign() in {-1,0,+1}; weights are
alpha*sign(w) with per-output-channel alpha. bf16 rounding of alpha is a pure
per-channel scale, which BatchNorm divides right back out, so conv in bf16 with
fp32 PSUM accumulation reproduces the fp32 reference exactly.

Bias/PReLU algebra is folded into per-channel scalars so the whole post-conv
chain is 4 fused passes:
  u = (c * g') + residual          [DVE scalar_tensor_tensor]
  r = Relu(u + Bp)                 [ACT, per-partition bias]
  q = (u * p) + (p*Bp [+ b23])     [DVE tensor_scalar dual-op]
  M = (r * (1-p)) + q              [GPSIMD scalar_tensor_tensor]
which equals prelu(u + Bp) [+ b23] since prelu(z) = (1-p)*relu(z) + p*z.
"""

import functools
import numpy as np

import concourse.bass as bass
import concourse.bacc as bacc
import concourse.tile as tile
from concourse import mybir

EPS = 1e-5
B, C, H = 8, 64, 128
NCORES = 8
PAD = 130          # padded row length (128 + 2)
ROWS = 66          # padded rows per half (64 + 2 halo)
XF = ROWS * PAD    # 8580 padded elements per partition
NF = 64 * 128      # 8192 dense elements per partition (half image)
NTOT = B * H * H   # 131072: count per channel for batch stats
TAPS = [(dy, dx) for dy in range(3) for dx in range(3)]

f32 = mybir.dt.float32
bf16 = mybir.dt.bfloat16
Alu = mybir.AluOpType
Act = mybir.ActivationFunctionType


def _post_bn_math(nc, sm, ar, gamma_col, beta_col, alpha_col, alphasq_col):
    """Tiny per-partition ([128,1]) math turning AllReduced integer count
    sums into per-channel scalars. The conv ran with pure sign(+-1) weights,
    so sums are exact integer counts; alpha is applied here in fp32.

    Writes sm cols: 2 = muk (mean in count units), 19 = G (gamma*rsqrt*alpha).
    The apply pass computes (k - muk)*G == gamma*(c - mu)*rsqrt(var+eps),
    matching the reference's operation order.
    """
    TT = nc.vector.tensor_tensor
    TS = nc.vector.tensor_scalar

    def c(i):
        return sm[:, i:i + 1]

    TT(out=c(0), in0=ar[:, 0:1], in1=ar[:, 2:3], op=Alu.add)      # S1 counts
    TT(out=c(1), in0=ar[:, 1:2], in1=ar[:, 3:4], op=Alu.add)      # S2 counts
    TS(c(2), c(0), 1.0 / NTOT, None, Alu.mult)                    # muk
    TS(c(3), c(1), 1.0 / NTOT, None, Alu.mult)                    # E[k^2]
    TS(c(4), c(2), c(2), None, Alu.mult)                          # muk^2
    TT(out=c(5), in0=c(3), in1=c(4), op=Alu.subtract)             # vark
    TT(out=c(6), in0=c(5), in1=alphasq_col, op=Alu.mult)          # var
    TS(c(6), c(6), EPS, None, Alu.add)                            # var+eps
    nc.scalar.activation(out=c(7), in_=c(6), func=Act.Sqrt)       # ~sqrt
    nc.vector.reciprocal(out=c(8), in_=c(7))                      # y0 ~ rsqrt
    # one Newton step: y = y0*(1.5 - 0.5*(var+eps)*y0^2)
    TS(c(9), c(8), c(8), None, Alu.mult)                          # y0^2
    TT(out=c(10), in0=c(9), in1=c(6), op=Alu.mult)                # v*y0^2
    TS(c(11), c(10), -0.5, 1.5, Alu.mult, Alu.add)                # 1.5-0.5*
    TT(out=c(12), in0=c(8), in1=c(11), op=Alu.mult)               # y (rsqrt)
    TT(out=c(13), in0=c(12), in1=gamma_col, op=Alu.mult)          # g'
    TT(out=c(19), in0=c(13), in1=alpha_col, op=Alu.mult)          # G = g'*alpha
    # fold the additive bias beta into the subtracted mean so the residual
    # add needs no per-channel scalar: (k - (muk - beta/G))*G = (k-muk)*G+beta
    nc.vector.reciprocal(out=c(14), in_=c(19))                    # 1/G
    TT(out=c(15), in0=beta_col, in1=c(14), op=Alu.mult)           # beta/G
    TT(out=c(20), in0=c(2), in1=c(15), op=Alu.subtract)           # muk'


def build_nc():
    nc = bacc.Bacc(None, num_devices=NCORES)

    x_t = nc.declare_dram_parameter("x", [C, H, H], f32, isOutput=False)
    w1_t = nc.declare_dram_parameter("w1t", [128, 9 * 128], bf16, isOutput=False)
    w2_t = nc.declare_dram_parameter("w2t", [128, 256], bf16, isOutput=False)
    pv_t = nc.declare_dram_parameter("pvec", [128, 20], f32, isOutput=False)
    out_t = nc.declare_dram_parameter("out", [C, H, H], f32, isOutput=True)

    RG = [list(range(NCORES))]
    # row-chunk boundaries for chunked full-image passes
    RCH = [(0, 16), (16, 32), (32, 48), (48, 64)]
    HCH = [(0, 32), (32, 64)]

    with tile.TileContext(nc, num_cores=NCORES) as tc:
        with tc.tile_pool(name="main", bufs=1) as pool, \
             tc.tile_pool(name="pp", bufs=2, space="PSUM") as pp, \
             tc.tile_pool(name="dp", bufs=1, space="DRAM") as dp:

            w1s = pool.tile([128, 9 * 128], bf16, name="w1s")
            w2s = pool.tile([128, 256], bf16, name="w2s")
            pv = pool.tile([128, 20], f32, name="pv")
            nc.sync.dma_start(out=w1s[:, :], in_=w1_t[:, :])
            nc.sync.dma_start(out=w2s[:, :], in_=w2_t[:, :])
            nc.sync.dma_start(out=pv[:, :], in_=pv_t[:, :])
            # preload the sqrt ACT table set now, not in the middle of the
            # AllReduce latency window
            warmsq = pool.tile([128, 1], f32, name="warmsq")
            nc.scalar.activation(out=warmsq[:, :], in_=pv[:, 12:13], func=Act.Sqrt)

            # ---- load x into padded/folded layout, split into two
            # overlapping row-tiles (global rows 0..35 / 32..66) so sign+conv
            # start while the second half still streams in ------------------
            xpA = pool.tile([128, 35 * PAD], f32, name="xpA", tag="A")
            xpB = pool.tile([128, 34 * PAD], f32, name="xpB", tag="A2")
            xa3 = xpA.rearrange("p (r c) -> p r c", c=PAD)
            xb3 = xpB.rearrange("p (r c) -> p r c", c=PAD)
            nc.vector.memset(xa3[:, :, 0:1], 0.0)
            nc.vector.memset(xa3[:, :, 129:130], 0.0)
            nc.vector.memset(xa3[0:64, 0:1, :], 0.0)     # half0 top halo
            nc.vector.memset(xb3[:, :, 0:1], 0.0)
            nc.vector.memset(xb3[:, :, 129:130], 0.0)
            nc.vector.memset(xb3[64:128, 33:34, :], 0.0)  # half1 bottom halo
            for (d0, d1) in [(0, 19), (19, 35)]:
                h0 = max(d0, 1)
                nc.sync.dma_start(out=xa3[0:64, h0:d1, 1:129],
                                  in_=x_t[:, h0 - 1:d1 - 1, :])
                nc.sync.dma_start(out=xa3[64:128, d0:d1, 1:129],
                                  in_=x_t[:, 63 + d0:63 + d1, :])
            # xpB local row r = global row 32+r; 3-row overlap loaded twice
            for (l0, l1) in [(0, 3), (3, 19), (19, 34)]:
                nc.sync.dma_start(out=xb3[0:64, l0:l1, 1:129],
                                  in_=x_t[:, 31 + l0:31 + l1, :])
                m1 = min(l1, 33)
                nc.sync.dma_start(out=xb3[64:128, l0:m1, 1:129],
                                  in_=x_t[:, 95 + l0:95 + m1, :])

            # ---- T1 = sign(x + b11), padded, bf16 -------------------------
            # T1 split into two overlapping row-tiles (global rows 0..35 and
            # 32..66) so conv1's first bank groups only depend on the first
            # sign pass instead of the whole activation tensor
            T1a = pool.tile([128, 35 * PAD], bf16, name="T1a", tag="B")
            T1b = pool.tile([128, 34 * PAD], bf16, name="T1b", tag="B2")
            Ta3 = T1a.rearrange("p (r c) -> p r c", c=PAD)
            Tb3 = T1b.rearrange("p (r c) -> p r c", c=PAD)
            wps = pp.tile([128, 512], f32, name="wps", tag="ps0")
            for half, (t3, src3) in enumerate([(Ta3, xa3), (Tb3, xb3)]):
                nc.scalar.activation(out=t3[:, :, :],
                                     in_=src3[:, 0:35 - half, :],
                                     func=Act.Sign, bias=pv[:, 0:1])
                # zero borders (sign(0+b11) may be nonzero there)
                nc.vector.memset(t3[:, :, 0:1], 0.0)
                nc.vector.memset(t3[:, :, 129:130], 0.0)
                if half == 0:
                    nc.vector.memset(t3[0:64, 0:1, :], 0.0)
                else:
                    nc.vector.memset(t3[64:128, 33:34, :], 0.0)
                # HAM warmup: keep the PE busy during the load/sign phase so
                # conv1 runs at 2.4 GHz from its first matmul
                for _ in range(16):
                    nc.tensor.matmul(wps[:, :], lhsT=w1s[:, 0:128],
                                     rhs=t3[:, 0:4, 0:128],
                                     start=True, stop=True,
                                     skip_group_check=True)

            # ---- conv1: 9 accumulating matmuls per 512-col chunk ----------
            c1p = pool.tile([128, NF], f32, name="c1p", tag="C")
            s1cols = pool.tile([128, 8], f32, name="s1cols")
            pk1 = pool.tile([128, 2], f32, name="pk1")
            s1sq = pool.tile([128, 8], f32, name="s1sq")
            sqs = pool.tile([128, 2 * 1024], bf16, name="sqs")
            # 8 double-chunks of 1024 cols; each psum tile spans 2 banks and
            # is drained by ONE copy + ONE square (halves ACT op count)
            for bg in range(4):
                pss = [pp.tile([128, 1024], f32, name=f"ps{bg}_{j}", tag=f"ps{j}")
                       for j in range(2)]
                # taps outer: the stationary operand is reused across the
                # group, so LDWEIGHTS amortizes and the PE gets dense
                # back-to-back matmul bursts (keeps HAM un-throttled)
                for t, (dy, dx) in enumerate(TAPS):
                    for j in range(2):
                        for h in range(2):
                            q = 4 * bg + 2 * j + h
                            src3 = Ta3 if bg < 2 else Tb3
                            r = 4 * q + dy - (0 if bg < 2 else 32)
                            nc.tensor.matmul(
                                pss[j][:, 512 * h:512 * (h + 1)],
                                lhsT=w1s[:, t * 128:(t + 1) * 128],
                                rhs=src3[:, r:r + 4, dx:dx + 128],
                                start=(t == 0), stop=(t == 8),
                                skip_group_check=True)
                for j in range(2):
                    dq = 2 * bg + j
                    nc.scalar.activation(out=c1p[:, 1024 * dq:1024 * (dq + 1)],
                                         in_=pss[j][:, :], func=Act.Copy,
                                         bias=0.0, scale=1.0,
                                         accum_out=s1cols[:, dq:dq + 1])
                    nc.scalar.activation(out=sqs[:, 1024 * j:1024 * (j + 1)],
                                         in_=pss[j][:, :], func=Act.Square,
                                         accum_out=s1sq[:, dq:dq + 1])

            nc.vector.tensor_reduce(out=pk1[:, 1:2], in_=s1sq[:, :],
                                    axis=mybir.AxisListType.X, op=Alu.add)
            nc.vector.tensor_reduce(out=pk1[:, 0:1], in_=s1cols[:, :],
                                    axis=mybir.AxisListType.X, op=Alu.add)

            # ---- AllReduce branch-1 stats ---------------------------------
            cc1i = dp.tile([128, 2], f32, name="cc1i")
            cc1o = dp.tile([128, 2], f32, name="cc1o", addr_space="Shared")
            nc.gpsimd.dma_start(out=cc1i[:, :], in_=pk1[:, :])
            nc.gpsimd.collective_compute("AllReduce", Alu.add, replica_groups=RG,
                                         ins=[cc1i[:, :].opt()],
                                         outs=[cc1o[:, :].opt()])
            ar1 = pool.tile([128, 4], f32, name="ar1")
            cc1r = cc1o[:, :].rearrange("(h c) s -> c h s", h=2)
            nc.sync.dma_start(out=ar1[0:64, :].rearrange("c (h s) -> c h s", h=2),
                              in_=cc1r)
            nc.sync.dma_start(out=ar1[64:128, :].rearrange("c (h s) -> c h s", h=2),
                              in_=cc1r)

            sm1 = pool.tile([128, 24], f32, name="sm1")
            # pv col2 = gamma1, col3 = beta1+b12, col4 = p1
            _post_bn_math(nc, sm1, ar1, pv[:, 2:3], pv[:, 3:4], pv[:, 11:12], pv[:, 12:13])

            # ---- branch-1 apply: bn = (k-muk)*G; u = bn+beta12+x;
            #      M1 = prelu(u) = max(u, p1*u) ----------------------------
            w1b = pool.tile([128, NF], f32, name="w1b", tag="D")
            w13 = w1b.rearrange("p (r c) -> p r c", c=128)
            u = pool.tile([128, NF], f32, name="u", tag="E")
            u3 = u.rearrange("p (r c) -> p r c", c=128)
            M1 = pool.tile([128, NF], f32, name="M1", tag="A")
            M13 = M1.rearrange("p (r c) -> p r c", c=128)
            for k, (r0, r1) in enumerate(HCH):
                sl = slice(128 * r0, 128 * r1)
                xsrc = xa3 if k == 0 else xb3
                nc.vector.tensor_scalar(w1b[:, sl], c1p[:, sl],
                                        sm1[:, 20:21], sm1[:, 19:20],
                                        Alu.subtract, Alu.mult)
                nc.vector.tensor_tensor(out=u3[:, r0:r1, :], in0=w13[:, r0:r1, :],
                                        in1=xsrc[:, 1:33, 1:129], op=Alu.add)
            for (r0, r1) in HCH:
                sl = slice(128 * r0, 128 * r1)
                nc.vector.scalar_tensor_tensor(out=M1[:, sl], in0=u[:, sl],
                                               scalar=pv[:, 4:5], in1=u[:, sl],
                                               op0=Alu.mult, op1=Alu.max)

            # ---- T2 = -deadzone_sign(M1 + b13 + b21), dense bf16 ----------
            # deadzone: |t| < theta -> 0 (theta ~ fp32 noise floor), so the
            # reference's exact-zero sign inputs stay zero. Negation is folded
            # into the sign of alpha2 on the host.
            # a = (M1 >= theta - sb2); b = (M1 <= -theta - sb2); the
            # subtraction T2 = a - b is absorbed into conv2 (two accumulating
            # matmuls: +W2^T a then -W2^T b)
            aT = pool.tile([128, NF], bf16, name="aT", tag="F")
            T2 = pool.tile([128, NF], bf16, name="T2", tag="B")
            for (r0, r1) in HCH:
                sl = slice(128 * r0, 128 * r1)
                nc.vector.tensor_scalar(aT[:, sl], M1[:, sl],
                                        pv[:, 15:16], None, Alu.is_ge)
                nc.vector.tensor_scalar(T2[:, sl], M1[:, sl],
                                        pv[:, 16:17], None, Alu.is_le)

            # ---- conv2 (1x1) ----------------------------------------------
            c2p = pool.tile([128, NF], f32, name="c2p", tag="C")
            s2cols = pool.tile([128, 8], f32, name="s2cols")
            pk2 = pool.tile([128, 2], f32, name="pk2")
            s2sq = pool.tile([128, 8], f32, name="s2sq")
            for bg in range(4):
                pss2 = [pp.tile([128, 1024], f32, name=f"ps2{bg}_{j}", tag=f"ps{j}")
                        for j in range(2)]
                for t in range(2):
                    for j in range(2):
                        for h in range(2):
                            q = 4 * bg + 2 * j + h
                            src = aT if t == 0 else T2
                            nc.tensor.matmul(
                                pss2[j][:, 512 * h:512 * (h + 1)],
                                lhsT=w2s[:, 128 * t:128 * (t + 1)],
                                rhs=src[:, 512 * q:512 * (q + 1)],
                                start=(t == 0), stop=(t == 1),
                                skip_group_check=True)
                for j in range(2):
                    dq = 2 * bg + j
                    nc.scalar.activation(out=c2p[:, 1024 * dq:1024 * (dq + 1)],
                                         in_=pss2[j][:, :], func=Act.Copy,
                                         bias=0.0, scale=1.0,
                                         accum_out=s2cols[:, dq:dq + 1])
                    nc.scalar.activation(out=sqs[:, 1024 * j:1024 * (j + 1)],
                                         in_=pss2[j][:, :], func=Act.Square,
                                         accum_out=s2sq[:, dq:dq + 1])
            nc.vector.tensor_reduce(out=pk2[:, 1:2], in_=s2sq[:, :],
                                    axis=mybir.AxisListType.X, op=Alu.add)
            nc.vector.tensor_reduce(out=pk2[:, 0:1], in_=s2cols[:, :],
                                    axis=mybir.AxisListType.X, op=Alu.add)

            # ---- AllReduce branch-2 stats ---------------------------------
            cc2i = dp.tile([128, 2], f32, name="cc2i")
            cc2o = dp.tile([128, 2], f32, name="cc2o", addr_space="Shared")
            nc.gpsimd.dma_start(out=cc2i[:, :], in_=pk2[:, :])
            nc.gpsimd.collective_compute("AllReduce", Alu.add, replica_groups=RG,
                                         ins=[cc2i[:, :].opt()],
                                         outs=[cc2o[:, :].opt()])
            ar2 = pool.tile([128, 4], f32, name="ar2")
            cc2r = cc2o[:, :].rearrange("(h c) s -> c h s", h=2)
            nc.sync.dma_start(out=ar2[0:64, :].rearrange("c (h s) -> c h s", h=2),
                              in_=cc2r)
            nc.sync.dma_start(out=ar2[64:128, :].rearrange("c (h s) -> c h s", h=2),
                              in_=cc2r)

            sm2 = pool.tile([128, 24], f32, name="sm2")
            # pv col6 = gamma2, col7 = beta2+b22+b13, col8 = p2, col10 = b23
            _post_bn_math(nc, sm2, ar2, pv[:, 6:7], pv[:, 7:8], pv[:, 13:14], pv[:, 14:15])

            # ---- branch-2 apply: u2 = (k2-muk2)*G2 + beta2eff + M1;
            #      out = max(u2+b23, p2*u2+b23) = prelu(u2)+b23 -------------
            w2b = pool.tile([128, NF], f32, name="w2b", tag="D")
            u2 = pool.tile([128, NF], f32, name="u2", tag="E")
            q2 = pool.tile([128, NF], f32, name="q2", tag="C")
            outv_a = pool.tile([128, NF // 2], f32, name="outv_a", tag="D")
            outv_b = pool.tile([128, NF // 2], f32, name="outv_b", tag="F")
            for (r0, r1) in HCH:
                sl = slice(128 * r0, 128 * r1)
                nc.vector.tensor_scalar(w2b[:, sl], c2p[:, sl],
                                        sm2[:, 20:21], sm2[:, 19:20],
                                        Alu.subtract, Alu.mult)
                nc.vector.tensor_tensor(out=u2[:, sl], in0=w2b[:, sl],
                                        in1=M1[:, sl], op=Alu.add)
                nc.vector.tensor_scalar(q2[:, sl], u2[:, sl],
                                        pv[:, 8:9], pv[:, 17:18],
                                        Alu.mult, Alu.add)
            for i, (r0, r1) in enumerate(HCH):
                sl = slice(128 * r0, 128 * r1)
                ov = outv_a if i == 0 else outv_b
                nc.vector.tensor_tensor(out=ov[:, :], in0=u2[:, sl],
                                        in1=q2[:, sl], op=Alu.max)

            # ---- store (per half-tile so DMA overlaps the second max) -----
            for i, ov in enumerate((outv_a, outv_b)):
                ov3 = ov.rearrange("p (r c) -> p r c", c=128)
                for (r0, r1) in [(0, 16), (16, 32)]:
                    nc.sync.dma_start(out=out_t[:, 32 * i + r0:32 * i + r1, :],
                                      in_=ov3[0:64, r0:r1, :])
                    nc.sync.dma_start(out=out_t[:, 64 + 32 * i + r0:64 + 32 * i + r1, :],
                                      in_=ov3[64:128, r0:r1, :])
    return nc


@functools.lru_cache(maxsize=1)
def get_nc():
    nc = build_nc()
    nc.finalize()   # run_bass_kernel_spmd/bass2jax expects a finalized program
    return nc


def _bf16(a):
    import ml_dtypes
    return a.astype(ml_dtypes.bfloat16)


def host_prep(inputs):
    """Build the small derived device inputs from the full problem inputs."""
    w1 = np.asarray(inputs["w1"], np.float32)     # (64, 16, 3, 3)
    w2 = np.asarray(inputs["w2"], np.float32)     # (64, 64, 1, 1)

    a1 = np.mean(np.abs(w1), axis=(1, 2, 3)).astype(np.float32)      # (64,)
    bw1 = np.asarray(_bf16(np.sign(w1)))          # pure +-1, exact in bf16
    a2 = np.mean(np.abs(w2), axis=(1, 2, 3)).astype(np.float32)
    bw2 = np.asarray(_bf16(np.sign(w2)))

    # conv1 lhsT: [K=128 (in-ch x half), 9 taps, M=128 (out-ch x half)]
    w1t = np.zeros((128, 9, 128), dtype=bw1.dtype)
    for o in range(C):
        g = o // 16
        for ir in range(16):
            i_abs = g * 16 + ir
            for t, (dy, dx) in enumerate(TAPS):
                val = bw1[o, ir, dy, dx]
                w1t[i_abs, t, o] = val
                w1t[64 + i_abs, t, 64 + o] = val
    w1t = w1t.reshape(128, 9 * 128)

    w2t = np.zeros((128, 256), dtype=bw2.dtype)
    w2t[0:64, 0:64] = bw2[:, :, 0, 0].T
    w2t[64:128, 64:128] = bw2[:, :, 0, 0].T
    w2t[:, 128:256] = -w2t[:, 0:128]

    def vec(name):
        return np.asarray(inputs[name], np.float32).reshape(C)

    cols = np.zeros((C, 20), np.float32)
    cols[:, 0] = vec("b11")
    cols[:, 1] = vec("b13") + vec("b21")
    cols[:, 2] = vec("bn1_gamma")
    cols[:, 3] = vec("bn1_beta") + vec("b12")
    cols[:, 4] = vec("p1")
    cols[:, 5] = 1.0 - vec("p1")
    cols[:, 6] = vec("bn2_gamma")
    cols[:, 7] = vec("bn2_beta") + vec("b22") + vec("b13") + vec("b23")
    cols[:, 8] = vec("p2")
    cols[:, 9] = 1.0 - vec("p2")
    cols[:, 10] = vec("b23")
    cols[:, 11] = a1
    cols[:, 12] = a1 * a1
    cols[:, 13] = a2
    cols[:, 14] = a2 * a2
    theta = 1e-6
    cols[:, 15] = theta - cols[:, 1]    # M1 >= theta - sb2  <=>  t >= theta
    cols[:, 16] = -theta - cols[:, 1]   # M1 <= -theta - sb2 <=>  t <= -theta
    cols[:, 17] = (1.0 - vec("p2")) * vec("b23")
    pvec = np.concatenate([cols, cols], axis=0)   # dup across halves -> [128,12]
    return w1t, w2t, pvec


def kernel(**inputs):
    from concourse.bass_utils import run_bass_kernel_spmd

    x = np.ascontiguousarray(np.asarray(inputs["x"], np.float32))   # (8,64,128,128)
    loss = np.asarray(inputs["loss"], np.float32)
    w1t, w2t, pvec = host_prep(inputs)

    nc = get_nc()
    in_maps = [{"x": np.ascontiguousarray(x[i]), "w1t": w1t, "w2t": w2t,
                "pvec": pvec} for i in range(NCORES)]
    res = run_bass_kernel_spmd(nc, in_maps, core_ids=list(range(NCORES)))
    out = np.stack([np.asarray(res.results[i]["out"]) for i in range(NCORES)])
    return out, loss
